# revision 1
# baseline (speedup 1.0000x reference)
# kernel.py — DyResConv_Inf (moe_routing) on 8 TRN2 NeuronCores.
#
# Reference computation:
#   r = routing(x)                      # (3, 768) sigmoid gates from global pools of x
#   w = sum_e r[e,o] * convs[e,o,:,:,:] # fused 3x3 conv weight synthesis
#   y = conv2d(x, w, stride 1, pad 1)   # (1, 768, 120, 120)
#
# One SPMD program on 8 cores; all per-core variation is input *data*
# prepared on the host. Core c (q = c//2 row-quarter, h = c%2 channel-half)
# computes y for out-channels [384h, 384h+384) x rows [30q, 30q+30).
#
# Routing head (latency-critical): per-core pool rows -> masked PE matmuls
# -> AllGather of [16,768] slot sums -> PE reconstruction + tiny routing
# net -> sigmoid gates r. Pool-mean divisions are folded into w_pw1 on the
# host. Junk PE matmuls bridge the collective wait to keep the PE p-state
# ramped.
#
# Weight synthesis: per k-tile diag-gate matmuls (lhsT = fp16 expert tile
# scaled x128, rhs = fp16 diag(r)) produce transposed gated weights in
# PSUM; ACT quantizes to fp8 e4m3 (wt_hi), DVE computes the residual and
# quantizes it too (wt_lo). No separate transpose pass, no DVE gating.
#
# Main conv in fp8 with MatmulPerfMode.DoubleRow (K=256 per instruction at
# 0.5 cyc/row): cin blocks are paired (g, g+3); x is shipped pre-paired
# and pre-split into x8 = e4m3(x) and xlo = e4m3(x - x8) by the host.
# Three DoubleRow streams per pair accumulate into the same PSUM:
#   wt_hi @ x8  +  wt_hi @ xlo  +  wt_lo @ x8     (w_lo*x_lo dropped)
# rhs windows are contiguous nr*122-wide spans (junk pad columns included,
# sliced away at the PSUM->SBUF copy, which also applies the 2^-7 descale).
#
import os

import numpy as np

os.environ.setdefault("MYCRO_LOCAL_CACHE", "1")

N_CORES = 8
C = 768          # in = out channels
H = W = 120
S = 48           # squeeze channels
E = 3            # experts
KK = 3           # kernel size
NB = 6           # channel blocks of 128 (768/128)
OBPC = 3         # o-blocks per core
QR = 30          # rows per quarter
PR = 15          # pool rows per core
NK = 54          # k-tiles: 9 taps x 6 cin-blocks
NDT = 6          # cv DMA tiles per (e, lob): 9 chunks each
DTC = 9          # chunks per cv DMA tile
XR = 33          # x rows per core incl. halo + junk row for contiguous spans
F32 = np.float32
WSC = 128.0      # weight scale folded into the fp16 expert stream

N_JUNK = int(os.environ.get("KBENCH_JUNK", "125"))

# conv output row chunks per pass: (local_row_start, n_rows)
PASS_CHUNKS = [
    [(0, 4), (4, 4), (8, 4), (12, 3)],
    [(15, 4), (19, 4), (23, 4), (27, 3)],
]


def _bicubic_mat(in_size, out_size):
    """PyTorch-style bicubic (a=-0.75), align_corners=False, border-replicate."""
    a = -0.75

    def k(x):
        x = abs(x)
        if x <= 1.0:
            return (a + 2) * x**3 - (a + 3) * x**2 + 1.0
        if x < 2.0:
            return a * x**3 - 5 * a * x**2 + 8 * a * x - 4 * a
        return 0.0

    M = np.zeros((out_size, in_size), dtype=F32)
    scale = in_size / out_size
    for j in range(out_size):
        src = (j + 0.5) * scale - 0.5
        i0 = int(np.floor(src))
        t = src - i0
        for m in range(-1, 3):
            idx = min(max(i0 + m, 0), in_size - 1)
            M[j, idx] += k(m - t)
    return M


def _slot_terms(blocksize, nblocks):
    """For each pool block: list of (core, slot) contributing partial sums."""
    terms = [[] for _ in range(nblocks)]
    for c in range(N_CORES):
        base = (PR * c) // blocksize
        for s in range(2):
            b = base + s
            if b >= nblocks:
                continue
            lo, hi = max(PR * c, blocksize * b), min(PR * c + PR, blocksize * (b + 1))
            if lo < hi:
                terms[b].append((c, s))
    return terms


_prog_cache = {}


def _get_program(iters=None):
    """Build (once per iters) the SPMD Bass/Tile program. Returns nc."""
    global _prog_cache
    if iters is None:
        iters = int(os.environ.get("KBENCH_ITERS", "1"))
    if iters in _prog_cache:
        return _prog_cache[iters]

    from contextlib import ExitStack

    import concourse.bass as bass
    import concourse.tile as tile
    from concourse import bacc, mybir

    f32 = mybir.dt.float32
    f16 = mybir.dt.float16
    fp8 = mybir.dt.float8e4
    bf16 = mybir.dt.bfloat16
    AX = mybir.AxisListType
    ALU = mybir.AluOpType
    ACT = mybir.ActivationFunctionType
    DR = mybir.MatmulPerfMode.DoubleRow

    nc = bacc.Bacc(
        "TRN2",
        target_bir_lowering=False,
        debug=False,
        enable_asserts=False,
        num_devices=N_CORES,
    )

    # ---- I/O tensors (per-core contents differ; shapes identical) ----
    x8_d = nc.dram_tensor("x8", [384, 2 * XR * 122], fp8, kind="ExternalInput").ap()
    xlo_d = nc.dram_tensor("xlo", [384, 2 * XR * 122], fp8, kind="ExternalInput").ap()
    xpt_d = nc.dram_tensor("xpt", [W, PR, C], fp8, kind="ExternalInput").ap()
    cv_d = nc.dram_tensor("cvs", [E, 384, NK * 128], f16, kind="ExternalInput").ap()
    cf32_d = nc.dram_tensor("cf32", [128, 18 * S + 25 + 128], f32,
                            kind="ExternalInput").ap()
    c48_d = nc.dram_tensor("c48", [S, 9 * 128 + 9 + 9 + 25], f32,
                           kind="ExternalInput").ap()
    sel_d = nc.dram_tensor("selmm", [128, 64], bf16, kind="ExternalInput").ap()
    mask_d = nc.dram_tensor("maskmm", [W, PR, 16], bf16,
                            kind="ExternalInput").ap()
    y_d = nc.dram_tensor("y_out", [384, QR, W], f32, kind="ExternalOutput").ap()

    with tile.TileContext(nc) as tc, ExitStack() as ctx:
        consts = ctx.enter_context(tc.tile_pool(name="consts", bufs=1))
        x8p = ctx.enter_context(tc.tile_pool(name="x8p", bufs=1))
        small = ctx.enter_context(tc.tile_pool(name="small", bufs=1))
        dram = ctx.enter_context(tc.tile_pool(name="dram", bufs=1, space="DRAM"))
        psy = ctx.enter_context(tc.tile_pool(name="psy", bufs=4, space="PSUM"))
        cvp = ctx.enter_context(tc.tile_pool(name="cvp", bufs=2))
        for _it in range(iters):
          rctx = ExitStack()
          xpp = rctx.enter_context(tc.tile_pool(name=f"xpp{_it}", bufs=1))
          psr = rctx.enter_context(
              tc.tile_pool(name=f"psr{_it}", bufs=2, space="PSUM"))

          # ---- pool-critical transfers first on the sync queue ----
          maskmm = xpp.tile([W, PR * 16], bf16, tag="maskmm")
          nc.sync.dma_start(maskmm[:], mask_d)

          xpt_sb = xpp.tile([W, PR * C], fp8, tag="xpt")
          xptv = xpt_sb[:].rearrange("w (r c) -> w r c", r=PR)
          xpt_dmas = [
              nc.sync.dma_start(xptv[:, r0:r1, :], xpt_d[:, r0:r1, :])
              for r0, r1 in ((0, 4), (4, 8), (8, 12), (12, PR))
          ]

          cf32 = consts.tile([128, 18 * S + 25 + 128], f32, tag="cf32")
          w1t = cf32[:, 0:18 * S]
          ones25 = cf32[:, 18 * S:18 * S + 25]
          ident = cf32[:, 18 * S + 25:]
          c48 = consts.tile([S, 9 * 128 + 9 + 9 + 25], f32, tag="c48")
          w2t = c48[:, 0:9 * 128]
          wdw1 = c48[:, 9 * 128:9 * 128 + 9]
          wdw2 = c48[:, 9 * 128 + 9:9 * 128 + 18]
          tkmm = c48[0:9, 9 * 128 + 18:9 * 128 + 43]
          selmm = consts.tile([128, 64], bf16, tag="selmm")

          # PE warmup: ramp the clock gate to full p-state before the
          # latency-critical pool matmuls (maskmm lands first, so junk
          # matmuls against it can start ~1.3us in; results never read)
          junkw = psr.tile([16, 128], f32, tag="pw", bufs=2, name=f"jw{_it}")
          for _w in range(40):
              nc.tensor.matmul(junkw[:], maskmm[:, 0:16], maskmm[:, 0:128],
                               start=True, stop=True, skip_group_check=True)
          junkp = psr.tile([128, 128], f32, tag="pw", bufs=2, name=f"jk{_it}")

          # ---- stage-1 pools as PE matmuls: payload[pat, c] ----
          ppay = [psr.tile([16, 384], f32, tag="prt", bufs=2,
                           name=f"ppay{h2}_{_it}") for h2 in range(2)]
          for row in range(PR):
              for h2 in range(2):
                  nc.tensor.matmul(
                      ppay[h2][:],
                      maskmm[:, row * 16:(row + 1) * 16],
                      xptv[:, row, h2 * 384:(h2 + 1) * 384],
                      start=(row == 0), stop=(row == PR - 1),
                  )
          payload = xpp.tile([16, C], bf16, tag="payload")
          for h2 in range(2):
              nc.scalar.activation(payload[:, h2 * 384:(h2 + 1) * 384],
                                   ppay[h2][:], ACT.Copy)
          actwarm = small.tile([1, 1], f32, tag="actwarm")
          nc.scalar.activation(actwarm[:], ident[0:1, 0:1], ACT.Sigmoid)
          nc.scalar.activation(actwarm[:], ident[0:1, 0:1], ACT.Relu)
          nc.scalar.activation(actwarm[:], ident[0:1, 0:1], ACT.Copy,
                               scale=ident[0:1, 0:1])

          # ---- AllGather the slot sums ----
          cc_in = dram.tile([16, C], bf16, tag="ccin")
          cc_out = dram.tile([N_CORES, 16, C], bf16, tag="ccout",
                             addr_space="Shared")
          cc_dma = nc.sync.dma_start(cc_in[:], payload[:])
          nc.gpsimd.collective_compute(
              "AllGather",
              ALU.bypass,
              replica_groups=[list(range(N_CORES))],
              ins=[cc_in[:].opt()],
              outs=[cc_out[:].opt()],
          )

          # gathered slot sums, (core,pat) on partitions: gpsimd queue right
          # behind the collective, so it dispatches the moment it completes
          g2pm = xpp.tile([128, C], bf16, tag="g2pm")
          nc.gpsimd.dma_start(
              g2pm[:], cc_out[:].rearrange("core pat c -> (core pat) c"))

          # x pairs on the gpsimd queue; mover order is set by deps below:
          # cc -> consts -> cv lob0 -> x8 -> cv lob1+2 -> xlo -> y
          x8_sb, xlo_sb, x8_dmas, xlo_dmas = [], [], [], []
          for g in range(3):
              t8 = x8p.tile([128, 2 * XR * 122], fp8, tag=f"x8_{g}")
              x8_dmas.append(
                  nc.gpsimd.dma_start(t8[:], x8_d[g * 128:(g + 1) * 128, :]))
              x8_sb.append(t8)
          for g in range(3):
              tl = x8p.tile([128, 2 * XR * 122], fp8, tag=f"xlo_{g}")
              xlo_dmas.append(
                  nc.gpsimd.dma_start(tl[:], xlo_d[g * 128:(g + 1) * 128, :]))
              xlo_sb.append(tl)

          cf32_dma = nc.sync.dma_start(cf32[:], cf32_d)
          tile.add_dep_helper(cf32_dma.ins, xpt_dmas[3].ins, sync=True,
                              reason="consts after xpt")
          nc.sync.dma_start(c48[:], c48_d)
          nc.sync.dma_start(selmm[:], sel_d)

          # junk matmuls bridge the collective wait: keep the PE busy (and
          # the p-state ramp warm) until the gathered payload lands
          junkb = psr.tile([16, 512], f32, tag="pw", bufs=2, name=f"jb{_it}")
          for _w in range(N_JUNK):
              nc.tensor.matmul(junkb[:], maskmm[:, 0:16], xpt_sb[:, 0:512],
                               start=True, stop=True, skip_group_check=True)

          # prefetch the expert-weight stream on the sync queue: its head
          # (cc_dma) only clears once the payload is out, so these never
          # race the routing-critical transfers
          cv_tiles = {}
          cv_dmas = []
          for lob in range(OBPC):
              for dt_i in range(NDT):
                  tl = []
                  for e in range(E):
                      t = cvp.tile([128, DTC * 128], f16, tag=f"cv{e}",
                                   bufs=6, name=f"cv{e}_{lob}_{dt_i}_{_it}")
                      cv_dmas.append(nc.sync.dma_start(
                          t[:],
                          cv_d[e, lob * 128:(lob + 1) * 128,
                               dt_i * DTC * 128:(dt_i + 1) * DTC * 128],
                      ))
                      tl.append(t)
                  cv_tiles[(lob, dt_i)] = tl
          # serial-mover priority (by first use): cv lob0, x8 (conv0 sweeps
          # 1-2), cv lob1+lob2 (synth), xlo (sweep 3 comes last)
          tile.add_dep_helper(x8_dmas[0].ins, cv_dmas[17].ins, sync=True,
                              reason="x8 after cv lob0")
          tile.add_dep_helper(xlo_dmas[0].ins, cv_dmas[53].ins, sync=True,
                              reason="xlo after cv lob2")

          # pool block sums via SEL matmuls
          stg = xpp.tile([32, C], f32, tag="stg")
          stg3 = xpp.tile([9, C], f32, tag="stg3")
          stgu = xpp.tile([25, C], f32, tag="stgu")
          for h2 in range(2):
              psel = psr.tile([32, 384], f32, tag="prt", bufs=2,
                              name=f"psel{h2}_{_it}")
              nc.tensor.matmul(psel[:], selmm[:][:, 0:32],
                               g2pm[:, h2 * 384:(h2 + 1) * 384],
                               start=True, stop=True)
              nc.scalar.activation(stg[:, h2 * 384:(h2 + 1) * 384], psel[:],
                                   ACT.Copy)
              psel3 = psr.tile([9, 384], f32, tag="prt", bufs=2,
                               name=f"psel3{h2}_{_it}")
              nc.tensor.matmul(psel3[:], selmm[:][:, 32:41],
                               g2pm[:, h2 * 384:(h2 + 1) * 384],
                               start=True, stop=True)
              nc.scalar.activation(stg3[:, h2 * 384:(h2 + 1) * 384], psel3[:],
                                   ACT.Copy)
          # bicubic 3->5 as a matmul in transposed space: a3uT = Tk.T @ a3T
          for h2 in range(2):
              pbic = psr.tile([25, 384], f32, tag="prt", bufs=2,
                              name=f"pbic{h2}_{_it}")
              nc.tensor.matmul(pbic[:], tkmm,
                               stg3[:, h2 * 384:(h2 + 1) * 384],
                               start=True, stop=True)
              nc.scalar.activation(stgu[:, h2 * 384:(h2 + 1) * 384], pbic[:],
                                   ACT.Copy)

          # transpose per pblock back to channel-major
          att = small.tile([128, 18 * 25], f32, tag="att")
          att5 = xpp.tile([128, NB * 32], f32, tag="att5")
          for p in range(NB):
              pt1 = psr.tile([128, 32], f32, tag="pw", bufs=2,
                             name=f"pt1_{p}_{_it}")
              nc.tensor.transpose(pt1[:], stg[:, p * 128:(p + 1) * 128],
                                  ident[0:32, 0:32])
              nc.scalar.activation(att5[:, p * 32:(p + 1) * 32], pt1[:], ACT.Copy)
              pt2 = psr.tile([128, 25], f32, tag="pw", bufs=2,
                             name=f"pt2_{p}_{_it}")
              nc.tensor.transpose(pt2[:], stgu[:, p * 128:(p + 1) * 128],
                                  ident[0:25, 0:25])
              nc.scalar.activation(att[:, (6 + p) * 25:(7 + p) * 25], pt2[:],
                                   ACT.Copy)
              # a1e block: ones * total (raw sums; scaling folded into w_pw1)
              nc.vector.tensor_scalar_mul(
                  att[:, p * 25:(p + 1) * 25], ones25,
                  att5[:, p * 32 + 25:p * 32 + 26])

          # ---- routing net ----
          ph = psr.tile([S, 25], f32, tag="prt", bufs=2)
          for j in range(18):
              rhs = (att5[:, (j - 12) * 32:(j - 12) * 32 + 25] if j >= 12
                     else att[:, j * 25:(j + 1) * 25])
              nc.tensor.matmul(
                  ph[:],
                  w1t[:, j * S:(j + 1) * S],
                  rhs,
                  start=(j == 0), stop=(j == 17),
              )
          hdd1 = xpp.tile([S, 25], f32, tag="hdd1")
          nc.scalar.activation(hdd1[:], ph[:], ACT.Relu)

          hdd2 = xpp.tile([S, 9], f32, tag="hdd2")
          h1v = hdd1[:].rearrange("s (p q) -> s p q", p=5)
          for uv in range(9):
              u, v = uv // 3, uv % 3
              if uv == 0:
                  nc.vector.tensor_scalar_mul(
                      hdd2[:].rearrange("s (p q) -> s p q", p=3),
                      h1v[:, u:u + 3, v:v + 3], wdw1[:, 0:1]
                  )
              else:
                  t9 = xpp.tile([S, 9], f32, tag="t9", name=f"t9_{uv}")
                  nc.vector.tensor_scalar_mul(
                      t9[:].rearrange("s (p q) -> s p q", p=3),
                      h1v[:, u:u + 3, v:v + 3], wdw1[:, uv:uv + 1]
                  )
                  nc.vector.tensor_tensor(
                      out=hdd2[:], in0=hdd2[:], in1=t9[:], op=ALU.add
                  )
          nc.scalar.activation(hdd2[:], hdd2[:], ACT.Relu)

          t9b = xpp.tile([S, 9], f32, tag="t9b")
          nc.vector.tensor_tensor(out=t9b[:], in0=hdd2[:], in1=wdw2[:], op=ALU.mult)
          hdd3 = xpp.tile([S, 1], f32, tag="hdd3")
          nc.vector.tensor_reduce(hdd3[:], t9b[:], axis=AX.X, op=ALU.add)
          nc.scalar.activation(hdd3[:], hdd3[:], ACT.Relu)

          pr = psr.tile([128, 9], f32, tag="prt", bufs=2)
          for m in range(9):
              nc.tensor.matmul(
                  pr[:, m:m + 1],
                  w2t[:, m * 128:(m + 1) * 128],
                  hdd3[:],
                  start=True, stop=True, skip_group_check=True,
              )
          r_sb = small.tile([128, 9], f32, tag="r_sb")
          nc.scalar.activation(r_sb[:], pr[:], ACT.Sigmoid)

          # fp16 diag-gate matrices for PE-side synthesis (col = e*3+lob)
          dall = small.tile([128, 9 * 128], f16, tag="dall")
          for col in range(9):
              nc.vector.tensor_scalar_mul(
                  dall[:, col * 128:(col + 1) * 128], ident[:],
                  r_sb[:, col:col + 1]
              )

          rctx.close()  # free routing-phase SBUF before the conv phase
          ictx = ExitStack()
          wtp = ictx.enter_context(tc.tile_pool(name=f"wtp{_it}", bufs=2))
          tqp = ictx.enter_context(tc.tile_pool(name=f"tqp{_it}", bufs=2))
          psw = ictx.enter_context(
              tc.tile_pool(name=f"psw{_it}", bufs=4, space="PSUM"))

          def synth(lob):
              """Gate+transpose+fp8-split weights for o-block lob."""
              wthi = wtp.tile([128, NK * 128], fp8, tag="wthi",
                              name=f"wthi{lob}_{_it}")
              wtlo = wtp.tile([128, NK * 128], fp8, tag="wtlo",
                              name=f"wtlo{lob}_{_it}")
              for dt_i in range(NDT):
                  cvt = cv_tiles[(lob, dt_i)]
                  for grp in range(3):           # 3 k-tiles per PSUM drain
                      k0 = dt_i * DTC + grp * 3
                      pw = psw.tile([128, 384], f32, tag="pws", bufs=4,
                                    name=f"pw{lob}_{k0}_{_it}")
                      for ci in range(3):
                          for e in range(E):
                              nc.tensor.matmul(
                                  pw[:, ci * 128:(ci + 1) * 128],
                                  cvt[e][:, (grp * 3 + ci) * 128:
                                          (grp * 3 + ci + 1) * 128],
                                  dall[:, (e * 3 + lob) * 128:
                                       (e * 3 + lob + 1) * 128],
                                  start=(e == 0), stop=(e == E - 1),
                                  skip_group_check=True,
                              )
                      hi_blk = wthi[:, k0 * 128:(k0 + 3) * 128]
                      nc.scalar.activation(hi_blk, pw[:], ACT.Copy)
                      nc.vector.tensor_tensor(
                          out=wtlo[:, k0 * 128:(k0 + 3) * 128],
                          in0=pw[:], in1=hi_blk, op=ALU.subtract)
              return wthi, wtlo

          def emit_mms(lob, hi5, lo5, chunks, pys, ci_sel=None):
              """DoubleRow matmul stream for the given chunks (or one chunk)."""
              for t in range(3):
                  for uv in range(9):
                      u, v = uv // 3, uv % 3
                      for g in range(3):
                          lhsT = hi5[:, uv, g] if t != 1 else lo5[:, uv, g]
                          xsb = x8_sb[g] if t != 2 else xlo_sb[g]
                          xv = xsb[:].rearrange("c (i n) -> c i n", i=2)
                          for ci, (r0, nr) in enumerate(chunks):
                              if ci_sel is not None and ci != ci_sel:
                                  continue
                              off = (r0 + u) * 122 + v
                              nc.tensor.matmul(
                                  pys[ci][:],
                                  lhsT,
                                  xv[:, :, off:off + nr * 122],
                                  start=(t == 0 and (uv, g) == (0, 0)),
                                  stop=(t == 2 and (uv, g) == (8, 2)),
                                  perf_mode=DR,
                              )

          def drain(lob, chunks, pys, ci):
              r0, nr = chunks[ci]
              ysb = small.tile(
                  [128, nr * W], f32, tag="ysb", bufs=3,
                  name=f"ysb{lob}_{r0}_{_it}",
              )
              pyv = pys[ci][:].rearrange("c (r w) -> c r w", r=nr)
              ysv = ysb[:].rearrange("c (r w) -> c r w", r=nr)
              nc.scalar.activation(ysv, pyv[:, :, 0:W], ACT.Copy,
                                   scale=1.0 / WSC)
              nc.sync.dma_start(
                  y_d[lob * 128:(lob + 1) * 128, r0:r0 + nr, :], ysb[:]
              )

          def conv(lob, wthi, wtlo, final=False):
              """Main 3x3 conv for o-block lob via fp8 DoubleRow matmuls."""
              hi5 = wthi[:].rearrange("c (uv i g o) -> c uv g i o",
                                      uv=9, i=2, g=3)
              lo5 = wtlo[:].rearrange("c (uv i g o) -> c uv g i o",
                                      uv=9, i=2, g=3)
              for pi, chunks in enumerate(PASS_CHUNKS):
                  pys = [
                      psy.tile([128, nr * 122], f32, tag="py",
                               name=f"py{lob}_{r0}_{_it}")
                      for (r0, nr) in chunks
                  ]
                  if final and pi == len(PASS_CHUNKS) - 1:
                      # last pass of the whole conv: finish chunk-by-chunk so
                      # the tail after the very last matmul is one chunk deep
                      for ci in range(len(chunks)):
                          emit_mms(lob, hi5, lo5, chunks, pys, ci_sel=ci)
                          drain(lob, chunks, pys, ci)
                  else:
                      emit_mms(lob, hi5, lo5, chunks, pys)
                      for ci in range(len(chunks)):
                          drain(lob, chunks, pys, ci)

          # s0, s1, c0, s2, c1, c2: synthesis tails hide under the prior conv
          wt0 = synth(0)
          wt1 = synth(1)
          conv(0, *wt0)
          wt2 = synth(2)
          conv(1, *wt1)
          conv(2, *wt2, final=True)
          ictx.close()

    nc.finalize()
    _prog_cache[iters] = nc
    return nc


def prepare_in_maps(x, convs, w_pw1, w_dw1, w_dw2, w_pw2):
    """Host-side slicing/layout prep. Returns list of 8 per-core input dicts."""
    import ml_dtypes

    x = np.asarray(x, dtype=F32)
    convs = np.asarray(convs, dtype=F32)
    w_pw1 = np.asarray(w_pw1, dtype=F32)
    w_dw1 = np.asarray(w_dw1, dtype=F32)
    w_dw2 = np.asarray(w_dw2, dtype=F32)
    w_pw2 = np.asarray(w_pw2, dtype=F32)
    e4m3 = ml_dtypes.float8_e4m3

    x0 = x[0]  # (768, 120, 120)
    # 123 padded rows: row 0 top pad, rows 1..120 image, 121 bottom pad,
    # 122 junk (contiguous-span overrun)
    xpad = np.zeros((C, 123, 122), dtype=F32)
    xpad[:, 1:H + 1, 1:W + 1] = x0

    # convs -> [e, o, (u, v, cin)], prescaled by WSC for the fp8 path
    cvr = np.ascontiguousarray(
        convs.transpose(0, 1, 3, 4, 2).reshape(E, C, KK * KK * C)
    ) * WSC

    # w_pw1 prescaled by pool-mean factors, transposed, pblock-major
    colscale = np.concatenate([
        np.full(C, 1.0 / (H * W), dtype=F32),
        np.full(C, 1.0 / 1600.0, dtype=F32),
        np.full(C, 1.0 / 576.0, dtype=F32),
    ])
    w1s = (w_pw1 * colscale[None, :]).astype(F32)          # (48, 2304)
    w1t = np.ascontiguousarray(
        w1s.T.reshape(18, 128, S).transpose(1, 0, 2).reshape(128, 18 * S)
    )

    M35 = _bicubic_mat(3, 5)                                # (5, 3)
    tkmm = np.ascontiguousarray(
        np.einsum("pi,qj->ijpq", M35, M35).reshape(9, 25)).astype(F32)
    sel = np.zeros((128, 64), dtype=F32)
    for b, terms in enumerate(_slot_terms(24, 5)):
        for (c_, s_) in terms:
            for z in range(5):
                sel[c_ * 16 + s_ * 8 + z, b * 5 + z] = 1.0
                sel[c_ * 16 + s_ * 8 + z, 25] = 1.0      # a1 = sum of all a5
    for b, terms in enumerate(_slot_terms(40, 3)):
        for (c_, s_) in terms:
            for z in range(3):
                sel[c_ * 16 + s_ * 8 + 5 + z, 32 + b * 3 + z] = 1.0
    sel = sel.astype(ml_dtypes.bfloat16)

    ones25 = np.ones((128, 25), dtype=F32)
    ident = np.eye(128, dtype=F32)
    wdw1 = np.ascontiguousarray(w_dw1.reshape(S, 9))
    wdw2 = np.ascontiguousarray(w_dw2.reshape(S, 9))

    in_maps = []
    for c in range(N_CORES):
        q, h = c // 2, c % 2
        xq = xpad[:, 30 * q:30 * q + XR, :]               # (768, 33, 122)
        x8q = xq.astype(e4m3)
        xloq = (xq - x8q.astype(F32)).astype(e4m3)
        # pair layout: row g*128+cc holds (i, hw) for cin blocks g and g+3
        x8a = np.empty((384, 2, XR * 122), dtype=e4m3)
        xloa = np.empty((384, 2, XR * 122), dtype=e4m3)
        for g in range(3):
            for i in range(2):
                blk = slice((g + 3 * i) * 128, (g + 3 * i + 1) * 128)
                x8a[g * 128:(g + 1) * 128, i] = x8q[blk].reshape(128, -1)
                xloa[g * 128:(g + 1) * 128, i] = xloq[blk].reshape(128, -1)
        x8a = np.ascontiguousarray(x8a.reshape(384, 2 * XR * 122))
        xloa = np.ascontiguousarray(xloa.reshape(384, 2 * XR * 122))
        xpt = np.ascontiguousarray(
            x0[:, PR * c:PR * (c + 1), :].transpose(2, 1, 0)
        ).astype(e4m3)                                     # (120, 15, 768)
        maskmm = np.zeros((PR, W, 16), dtype=F32)
        for r_ in range(PR):
            grow = PR * c + r_
            for col in range(W):
                pc5, pc3 = col // 24, col // 40
                for s_ in range(2):
                    if grow // 24 == (PR * c) // 24 + s_:
                        maskmm[r_, col, s_ * 8 + pc5] = 1.0
                    if grow // 40 == (PR * c) // 40 + s_:
                        maskmm[r_, col, s_ * 8 + 5 + pc3] = 1.0
        maskmm = np.ascontiguousarray(
            maskmm.transpose(1, 0, 2)).astype(ml_dtypes.bfloat16)
        cvs = np.ascontiguousarray(
            cvr[:, 384 * h:384 * (h + 1), :]).astype(np.float16)
        w2t = np.empty((S, 9 * 128), dtype=F32)
        for e in range(E):
            for lob in range(OBPC):
                rows = slice(e * C + (3 * h + lob) * 128,
                             e * C + (3 * h + lob) * 128 + 128)
                w2t[:, (e * 3 + lob) * 128:(e * 3 + lob + 1) * 128] = w_pw2[rows, :].T
        cf32 = np.concatenate([w1t, ones25, ident], axis=1)
        c48 = np.concatenate(
            [w2t, wdw1, wdw2,
             np.concatenate([tkmm, np.zeros((S - 9, 25), dtype=F32)], axis=0)],
            axis=1)
        in_maps.append({
            "x8": x8a, "xlo": xloa, "xpt": xpt, "cvs": cvs, "cf32": cf32,
            "c48": c48, "selmm": sel, "maskmm": maskmm,
        })
    return in_maps


def reassemble(outs):
    """outs: list of 8 dicts with 'y_out' (384, 30, 120) -> (1, 768, 120, 120)."""
    y = np.empty((1, C, H, W), dtype=F32)
    for c in range(N_CORES):
        q, h = c // 2, c % 2
        y[0, 384 * h:384 * (h + 1), 30 * q:30 * (q + 1), :] = outs[c]["y_out"]
    return y


last_results = None  # BassKernelResults from the most recent run (for test.py)


def kernel(x, convs, w_pw1, w_dw1, w_dw2, w_pw2):
    global last_results
    from concourse import bass_utils

    nc = _get_program()
    in_maps = prepare_in_maps(x, convs, w_pw1, w_dw1, w_dw2, w_pw2)
    trace = bool(int(os.environ.get("KBENCH_TRACE", "0")))
    res = bass_utils.run_bass_kernel_spmd(
        nc, in_maps, core_ids=list(range(N_CORES)), trace=trace,
    )
    last_results = res
    return reassemble(res.results)



# revision 2
# speedup vs baseline: 1.0490x; 1.0490x over previous
# kernel2.py — DyResConv_Inf via F(2x2,3x3) Winograd on 8 TRN2 NeuronCores.
#
# Reference computation:
#   r = routing(x)                      # (3, 768) sigmoid gates from global pools of x
#   w = sum_e r[e,o] * convs[e,o,:,:,:] # fused 3x3 conv weight synthesis
#   y = conv2d(x, w, stride 1, pad 1)   # (1, 768, 120, 120)
#
# One SPMD program on 8 cores; core c (q = c//2 row-quarter, h = c%2
# channel-half) computes y for out-channels [384h, 384h+384) x rows
# [30q, 30q+30) (= Winograd tile strips [15q, 15q+15)).
#
# Routing head: identical to the direct-conv kernel (pool rows -> masked PE
# matmuls -> AllGather -> reconstruction + tiny net -> sigmoid gates).
#
# Weight synthesis: experts are pre-transformed on the host to the Winograd
# domain (scale-free G1 = [[1,0,0],[1,1,1],[1,-1,1],[0,0,1]]; the 1/2 plane
# scales are folded into the input transform), f16, x128. Device gates them
# per plane with diag(r) matmuls (output = transposed [cin, o] tiles), then
# ACT quantizes to fp8 e4m3 (wt_hi) and DVE computes + quantizes the
# residual (wt_lo).
#
# Main conv: 16 independent per-plane GEMMs in fp8 DoubleRow (cin blocks
# paired (g, g+3)), 3 streams accumulating into the same PSUM:
#   wt_hi @ d8  +  wt_lo @ d8  +  wt_hi @ dlo      (w_lo*d_lo dropped)
# d8/dlo are the host-side Winograd transform of x, split into e4m3 hi/lo.
# Per output chunk (= one tile strip: 2 out rows, 61 tile cols incl 1 junk),
# 16x9 DR matmuls accumulate m-planes in PSUM; the inverse transform
# (A1T = [[1,1,1,0],[0,1,-1,-1]]) runs as strided tensor_tensor ops on
# DVE (stage W, PSUM->SBUF) and Pool (stage H, SBUF->f16 y, x128 scaled;
# descaled on the host).
import os

import numpy as np

os.environ.setdefault("MYCRO_LOCAL_CACHE", "1")

N_CORES = 8
C = 768          # in = out channels
H = W = 120
S = 48           # squeeze channels
E = 3            # experts
NB = 6           # channel blocks of 128 (768/128)
OBPC = 3         # o-blocks per core
QR = 30          # output rows per quarter
PR = 15          # pool rows per core
NP = 16          # Winograd planes
NT = 15          # tile strips per core (2 out rows each)
TW = 61          # tile cols per strip incl 1 junk col
F32 = np.float32
WSC = 128.0      # weight scale folded into the f16 expert stream

N_JUNK = int(os.environ.get("KBENCH_JUNK", "0"))
N_WARM = int(os.environ.get("KBENCH_WARM", "0"))
GJUNK = int(os.environ.get("KBENCH_GJUNK", "0"))
QJUNK = int(os.environ.get("KBENCH_QJUNK", "0"))
PYB = int(os.environ.get("KBENCH_PYB", "3"))
CVB = int(os.environ.get("KBENCH_CVB", "3"))
JUNK4 = int(os.environ.get("KBENCH_JUNK4", "8"))
DDB = int(os.environ.get("KBENCH_DDB", "4"))


def _bicubic_mat(in_size, out_size):
    """PyTorch-style bicubic (a=-0.75), align_corners=False, border-replicate."""
    a = -0.75

    def k(x):
        x = abs(x)
        if x <= 1.0:
            return (a + 2) * x**3 - (a + 3) * x**2 + 1.0
        if x < 2.0:
            return a * x**3 - 5 * a * x**2 + 8 * a * x - 4 * a
        return 0.0

    M = np.zeros((out_size, in_size), dtype=F32)
    scale = in_size / out_size
    for j in range(out_size):
        src = (j + 0.5) * scale - 0.5
        i0 = int(np.floor(src))
        t = src - i0
        for m in range(-1, 3):
            idx = min(max(i0 + m, 0), in_size - 1)
            M[j, idx] += k(m - t)
    return M


def _slot_terms(blocksize, nblocks):
    """For each pool block: list of (core, slot) contributing partial sums."""
    terms = [[] for _ in range(nblocks)]
    for c in range(N_CORES):
        base = (PR * c) // blocksize
        for s in range(2):
            b = base + s
            if b >= nblocks:
                continue
            lo, hi = max(PR * c, blocksize * b), min(PR * c + PR, blocksize * (b + 1))
            if lo < hi:
                terms[b].append((c, s))
    return terms


_prog_cache = {}


def _get_program(iters=None):
    """Build (once per iters) the SPMD Bass/Tile program. Returns nc."""
    global _prog_cache
    if iters is None:
        iters = int(os.environ.get("KBENCH_ITERS", "1"))
    if iters in _prog_cache:
        return _prog_cache[iters]

    from contextlib import ExitStack

    import concourse.bass as bass
    import concourse.tile as tile
    from concourse import bacc, mybir

    f32 = mybir.dt.float32
    f16 = mybir.dt.float16
    fp8 = mybir.dt.float8e4
    bf16 = mybir.dt.bfloat16
    AX = mybir.AxisListType
    ALU = mybir.AluOpType
    ACT = mybir.ActivationFunctionType
    DR = mybir.MatmulPerfMode.DoubleRow

    nc = bacc.Bacc(
        "TRN2",
        target_bir_lowering=False,
        debug=False,
        enable_asserts=False,
        num_devices=N_CORES,
    )

    # ---- I/O tensors (per-core contents differ; shapes identical) ----
    # d: Winograd-transformed x, hi/lo fp8: [g-block rows, chunk, st, i, pl, TW]
    dd_d = nc.dram_tensor("dd", [384, NT * 2 * 2 * NP * TW], fp8,
                          kind="ExternalInput").ap()
    xpt_d = nc.dram_tensor("xpt", [W, PR, C], fp8, kind="ExternalInput").ap()
    # pre-transformed experts: [e, o(384), plane(16) x cin(768)] f16 x128
    cv_d = nc.dram_tensor("cvs", [E, 384, NP * C], f16, kind="ExternalInput").ap()
    cf32_d = nc.dram_tensor("cf32", [128, 18 * S + 25 + 128], f32,
                            kind="ExternalInput").ap()
    c48_d = nc.dram_tensor("c48", [S, 9 * 128 + 9 + 9 + 25], f32,
                           kind="ExternalInput").ap()
    sel_d = nc.dram_tensor("selmm", [128, 64], bf16, kind="ExternalInput").ap()
    mask_d = nc.dram_tensor("maskmm", [W, PR, 16], bf16,
                            kind="ExternalInput").ap()
    y_d = nc.dram_tensor("y_out", [384, QR, 128], f16, kind="ExternalOutput").ap()

    with tile.TileContext(nc) as tc, ExitStack() as ctx:
        consts = ctx.enter_context(tc.tile_pool(name="consts", bufs=1))
        small = ctx.enter_context(tc.tile_pool(name="small", bufs=1))
        dram = ctx.enter_context(tc.tile_pool(name="dram", bufs=1, space="DRAM"))
        cvp = ctx.enter_context(tc.tile_pool(name="cvp", bufs=2))
        ddp = ctx.enter_context(tc.tile_pool(name="ddp", bufs=2))
        for _it in range(iters):
          rctx = ExitStack()
          xpp = rctx.enter_context(tc.tile_pool(name=f"xpp{_it}", bufs=1))
          psr = rctx.enter_context(
              tc.tile_pool(name=f"psr{_it}", bufs=2, space="PSUM"))

          # ---- pool-critical transfers first on the sync queue ----
          maskmm = xpp.tile([W, PR * 16], bf16, tag="maskmm")
          nc.sync.dma_start(maskmm[:], mask_d)

          xpt_sb = xpp.tile([W, PR * C], fp8, tag="xpt")
          xptv = xpt_sb[:].rearrange("w (r c) -> w r c", r=PR)
          xpt_dmas = [
              nc.sync.dma_start(xptv[:, r0:r1, :], xpt_d[:, r0:r1, :])
              for r0, r1 in ((0, 4), (4, 8), (8, 12), (12, PR))
          ]

          # d chunks on the ACT queue (issued before any routing ACT work) so
          # cv-buffer stalls on the sync queue never block them; 3-strip
          # groups keep the DGE-gen cost small
          dd_tiles = {}
          dd_dmas = []
          CHB = 2 * 2 * NP * TW  # elems per (g, strip) per partition
          for grp in range(NT // 3):
              for g in range(3):
                  t = ddp.tile([128, 3 * CHB], fp8, tag=f"dd{g}",
                               bufs=2, name=f"dd{g}_{grp}_{_it}")
                  dd_dmas.append(nc.scalar.dma_start(
                      t[:], dd_d[g * 128:(g + 1) * 128,
                                 grp * 3 * CHB:(grp + 1) * 3 * CHB]))
                  dd_tiles[(grp, g)] = t

          cf32 = consts.tile([128, 18 * S + 25 + 128], f32, tag="cf32")
          w1t = cf32[:, 0:18 * S]
          ones25 = cf32[:, 18 * S:18 * S + 25]
          ident = cf32[:, 18 * S + 25:]
          c48 = consts.tile([S, 9 * 128 + 9 + 9 + 25], f32, tag="c48")
          w2t = c48[:, 0:9 * 128]
          wdw1 = c48[:, 9 * 128:9 * 128 + 9]
          wdw2 = c48[:, 9 * 128 + 9:9 * 128 + 18]
          tkmm = c48[0:9, 9 * 128 + 18:9 * 128 + 43]
          selmm = consts.tile([128, 64], bf16, tag="selmm")

          # PE warmup: ramp the clock gate to full p-state before the
          # latency-critical pool matmuls
          junkw = psr.tile([16, 128], f32, tag="pw", bufs=2, name=f"jw{_it}")
          for _w in range(N_WARM):
              nc.tensor.matmul(junkw[:], maskmm[:, 0:16], maskmm[:, 0:128],
                               start=True, stop=True, skip_group_check=True)

          # ---- stage-1 pools as PE matmuls: payload[pat, c] ----
          ppay = [psr.tile([16, 384], f32, tag="prt", bufs=2,
                           name=f"ppay{h2}_{_it}") for h2 in range(2)]
          for row in range(PR):
              for h2 in range(2):
                  nc.tensor.matmul(
                      ppay[h2][:],
                      maskmm[:, row * 16:(row + 1) * 16],
                      xptv[:, row, h2 * 384:(h2 + 1) * 384],
                      start=(row == 0), stop=(row == PR - 1),
                  )
          payload = xpp.tile([16, C], bf16, tag="payload")
          for h2 in range(2):
              nc.scalar.activation(payload[:, h2 * 384:(h2 + 1) * 384],
                                   ppay[h2][:], ACT.Copy)
          actwarm = small.tile([1, 1], f32, tag="actwarm")
          nc.scalar.activation(actwarm[:], ident[0:1, 0:1], ACT.Sigmoid)
          nc.scalar.activation(actwarm[:], ident[0:1, 0:1], ACT.Relu)
          nc.scalar.activation(actwarm[:], ident[0:1, 0:1], ACT.Copy,
                               scale=ident[0:1, 0:1])

          # ---- AllGather the slot sums ----
          cc_in = dram.tile([16, C], bf16, tag="ccin")
          cc_out = dram.tile([N_CORES, 16, C], bf16, tag="ccout",
                             addr_space="Shared")
          cc_dma = nc.scalar.dma_start(cc_in[:], payload[:])
          nc.gpsimd.collective_compute(
              "AllGather",
              ALU.bypass,
              replica_groups=[list(range(N_CORES))],
              ins=[cc_in[:].opt()],
              outs=[cc_out[:].opt()],
          )

          # gathered slot sums, (core,pat) on partitions
          g2pm = xpp.tile([128, C], bf16, tag="g2pm")
          nc.gpsimd.dma_start(
              g2pm[:], cc_out[:].rearrange("core pat c -> (core pat) c"))

          cf32_dma = nc.sync.dma_start(cf32[:], cf32_d)
          tile.add_dep_helper(cf32_dma.ins, xpt_dmas[3].ins, sync=True,
                              reason="consts after xpt")
          nc.sync.dma_start(c48[:], c48_d)
          nc.sync.dma_start(selmm[:], sel_d)

          # ---- expert-weight stream (sync queue, behind routing-critical
          # transfers): per (lob, e, plane-quad) tiles; then d chunks ----
          cv_tiles = {}
          cv_dmas = []
          for lob in range(OBPC):
              for pq in range(4):
                  for e in range(E):
                      t = cvp.tile([128, 4 * C], f16, tag=f"cv{e}",
                                   bufs=CVB, name=f"cv{e}_{lob}_{pq}_{_it}")
                      cv_dmas.append(nc.sync.dma_start(
                          t[:],
                          cv_d[e, lob * 128:(lob + 1) * 128,
                               pq * 4 * C:(pq + 1) * 4 * C],
                      ))
                      cv_tiles[(lob, pq, e)] = t

          # dd transfers yield to the routing-critical + first expert loads;
          # late groups yield to the cv stream (gating is cv-paced)
          tile.add_dep_helper(dd_dmas[0].ins, xpt_dmas[3].ins, sync=True,
                              reason="dd grp0 after xpt")
          tile.add_dep_helper(dd_dmas[3].ins, cv_dmas[11].ins, sync=True,
                              reason="dd grp1 after cv l0")
          tile.add_dep_helper(dd_dmas[6].ins, cv_dmas[23].ins, sync=True,
                              reason="dd grp2 after cv l1")
          tile.add_dep_helper(dd_dmas[9].ins, cv_dmas[29].ins, sync=True,
                              reason="dd grp3 after cv l2q1")
          tile.add_dep_helper(dd_dmas[12].ins, cv_dmas[35].ins, sync=True,
                              reason="dd grp4 after cv l2q3")

          # junk matmuls bridge the collective wait
          junkb = psr.tile([16, 512], f32, tag="pw", bufs=2, name=f"jb{_it}")
          for _w in range(N_JUNK):
              nc.tensor.matmul(junkb[:], maskmm[:, 0:16], xpt_sb[:, 0:512],
                               start=True, stop=True, skip_group_check=True)

          # pool block sums via SEL matmuls
          stg = xpp.tile([32, C], f32, tag="stg")
          stg3 = xpp.tile([9, C], f32, tag="stg3")
          stgu = xpp.tile([25, C], f32, tag="stgu")
          for h2 in range(2):
              psel = psr.tile([32, 384], f32, tag="prt", bufs=2,
                              name=f"psel{h2}_{_it}")
              nc.tensor.matmul(psel[:], selmm[:][:, 0:32],
                               g2pm[:, h2 * 384:(h2 + 1) * 384],
                               start=True, stop=True)
              nc.scalar.activation(stg[:, h2 * 384:(h2 + 1) * 384], psel[:],
                                   ACT.Copy)
              psel3 = psr.tile([9, 384], f32, tag="prt", bufs=2,
                               name=f"psel3{h2}_{_it}")
              nc.tensor.matmul(psel3[:], selmm[:][:, 32:41],
                               g2pm[:, h2 * 384:(h2 + 1) * 384],
                               start=True, stop=True)
              nc.scalar.activation(stg3[:, h2 * 384:(h2 + 1) * 384], psel3[:],
                                   ACT.Copy)
          # bicubic 3->5 as a matmul in transposed space
          for h2 in range(2):
              pbic = psr.tile([25, 384], f32, tag="prt", bufs=2,
                              name=f"pbic{h2}_{_it}")
              nc.tensor.matmul(pbic[:], tkmm,
                               stg3[:, h2 * 384:(h2 + 1) * 384],
                               start=True, stop=True)
              nc.scalar.activation(stgu[:, h2 * 384:(h2 + 1) * 384], pbic[:],
                                   ACT.Copy)

          # transpose per pblock back to channel-major
          att = small.tile([128, 18 * 25], f32, tag="att")
          att5 = xpp.tile([128, NB * 32], f32, tag="att5")
          for p in range(NB):
              pt1 = psr.tile([128, 32], f32, tag="pw", bufs=2,
                             name=f"pt1_{p}_{_it}")
              nc.tensor.transpose(pt1[:], stg[:, p * 128:(p + 1) * 128],
                                  ident[0:32, 0:32])
              nc.scalar.activation(att5[:, p * 32:(p + 1) * 32], pt1[:], ACT.Copy)
              pt2 = psr.tile([128, 25], f32, tag="pw", bufs=2,
                             name=f"pt2_{p}_{_it}")
              nc.tensor.transpose(pt2[:], stgu[:, p * 128:(p + 1) * 128],
                                  ident[0:25, 0:25])
              nc.scalar.activation(att[:, (6 + p) * 25:(7 + p) * 25], pt2[:],
                                   ACT.Copy)
              nc.vector.tensor_scalar_mul(
                  att[:, p * 25:(p + 1) * 25], ones25,
                  att5[:, p * 32 + 25:p * 32 + 26])

          # ---- routing net ----
          ph = psr.tile([S, 25], f32, tag="prt", bufs=2)
          for j in range(18):
              rhs = (att5[:, (j - 12) * 32:(j - 12) * 32 + 25] if j >= 12
                     else att[:, j * 25:(j + 1) * 25])
              nc.tensor.matmul(
                  ph[:],
                  w1t[:, j * S:(j + 1) * S],
                  rhs,
                  start=(j == 0), stop=(j == 17),
              )
          hdd1 = xpp.tile([S, 25], f32, tag="hdd1")
          nc.scalar.activation(hdd1[:], ph[:], ACT.Relu)

          hdd2 = xpp.tile([S, 9], f32, tag="hdd2")
          h1v = hdd1[:].rearrange("s (p q) -> s p q", p=5)
          for uv in range(9):
              u, v = uv // 3, uv % 3
              if uv == 0:
                  nc.vector.tensor_scalar_mul(
                      hdd2[:].rearrange("s (p q) -> s p q", p=3),
                      h1v[:, u:u + 3, v:v + 3], wdw1[:, 0:1]
                  )
              else:
                  t9 = xpp.tile([S, 9], f32, tag="t9", name=f"t9_{uv}")
                  nc.vector.tensor_scalar_mul(
                      t9[:].rearrange("s (p q) -> s p q", p=3),
                      h1v[:, u:u + 3, v:v + 3], wdw1[:, uv:uv + 1]
                  )
                  nc.vector.tensor_tensor(
                      out=hdd2[:], in0=hdd2[:], in1=t9[:], op=ALU.add
                  )
          nc.scalar.activation(hdd2[:], hdd2[:], ACT.Relu)

          t9b = xpp.tile([S, 9], f32, tag="t9b")
          nc.vector.tensor_tensor(out=t9b[:], in0=hdd2[:], in1=wdw2[:], op=ALU.mult)
          hdd3 = xpp.tile([S, 1], f32, tag="hdd3")
          nc.vector.tensor_reduce(hdd3[:], t9b[:], axis=AX.X, op=ALU.add)
          nc.scalar.activation(hdd3[:], hdd3[:], ACT.Relu)

          for _w in range(JUNK4):
              nc.tensor.matmul(junkb[:], maskmm[:, 0:16], xpt_sb[:, 0:512],
                               start=True, stop=True, skip_group_check=True)
          pr = psr.tile([128, 9], f32, tag="prt", bufs=2)
          for m in range(9):
              nc.tensor.matmul(
                  pr[:, m:m + 1],
                  w2t[:, m * 128:(m + 1) * 128],
                  hdd3[:],
                  start=True, stop=True, skip_group_check=True,
              )
          r_sb = small.tile([128, 9], f32, tag="r_sb")
          nc.scalar.activation(r_sb[:], pr[:], ACT.Sigmoid)

          # f16 diag-gate matrices for PE-side synthesis (col = e*3+lob)
          dall = small.tile([128, 9 * 128], f16, tag="dall")
          for col in range(9):
              nc.vector.tensor_scalar_mul(
                  dall[:, col * 128:(col + 1) * 128], ident[:],
                  r_sb[:, col:col + 1]
              )

          # bridge the routing-net serial tail (ACT/DVE ping-pong to gates)
          for _w in range(GJUNK):
              nc.tensor.matmul(junkb[:], maskmm[:, 0:16], xpt_sb[:, 0:512],
                               start=True, stop=True, skip_group_check=True)

          rctx.close()  # free routing-phase SBUF/PSUM before the conv phase
          ictx = ExitStack()
          wtp = ictx.enter_context(tc.tile_pool(name=f"wtp{_it}", bufs=3))
          vbuf = ictx.enter_context(tc.tile_pool(name=f"vbuf{_it}", bufs=2))
          psy = ictx.enter_context(
              tc.tile_pool(name=f"psy{_it}", bufs=4, space="PSUM"))
          psw = ictx.enter_context(
              tc.tile_pool(name=f"psw{_it}", bufs=2, space="PSUM"))

          junkg = (psy.tile([16, 128], f32, tag="junkg", bufs=1,
                            name=f"jg{_it}") if QJUNK else None)

          def synth(lob):
              """Gate the pre-transformed experts for o-block lob; fp8 split.
              Weight layout: [cin-in-block, (plane, i, g, o)]. Junk matmuls
              after each quad keep the PE p-state hot across cv-DMA stalls."""
              wthi = wtp.tile([128, NP * 2 * 3 * 128], fp8, tag="wthi",
                              name=f"wthi{lob}_{_it}")
              wtlo = wtp.tile([128, NP * 2 * 3 * 128], fp8, tag="wtlo",
                              name=f"wtlo{lob}_{_it}")
              hv = wthi[:].rearrange("c (pl b o) -> c pl b o", pl=NP, b=6)
              lv = wtlo[:].rearrange("c (pl b o) -> c pl b o", pl=NP, b=6)
              for pl in range(NP):
                  cvq = [cv_tiles[(lob, pl // 4, e)][:].rearrange(
                      "o (p c) -> o p c", p=4) for e in range(E)]
                  pw = psw.tile([128, 768], f32, tag="pws", bufs=2,
                                name=f"pw{lob}_{pl}_{_it}")
                  for b in range(6):
                      for e in range(E):
                          nc.tensor.matmul(
                              pw[:, b * 128:(b + 1) * 128],
                              cvq[e][:, pl % 4, b * 128:(b + 1) * 128],
                              dall[:, (e * 3 + lob) * 128:
                                   (e * 3 + lob + 1) * 128],
                              start=(e == 0), stop=(e == E - 1),
                              skip_group_check=True,
                          )
                  hi_blk = hv[:, pl]
                  nc.scalar.activation(hi_blk, pw[:], ACT.Copy)
                  nc.vector.tensor_tensor(out=lv[:, pl], in0=pw[:],
                                          in1=hi_blk, op=ALU.subtract)
                  if pl % 4 == 3:
                      for _w in range(QJUNK):
                          nc.tensor.matmul(junkg[:], dall[:, 0:16],
                                           dall[:, 0:128], start=True,
                                           stop=True, skip_group_check=True)
              return wthi, wtlo

          def conv_chunk(lob, ch, wthi, wtlo):
              """One tile strip of the Winograd conv for o-block lob: 16
              plane-GEMMs in fp8 DR + inverse transform (DVE stage-W,
              Pool stage-H)."""
              hv = wthi[:].rearrange("c (pl i g o) -> c pl i g o",
                                     pl=NP, i=2, g=3)
              lv = wtlo[:].rearrange("c (pl i g o) -> c pl i g o",
                                     pl=NP, i=2, g=3)
              ddv = [dd_tiles[(ch // 3, g)][:,
                                            (ch % 3) * CHB:(ch % 3 + 1) * CHB]
                     .rearrange("c (st i pl t) -> c st i pl t", st=2, i=2,
                                pl=NP)
                     for g in range(3)]
              # PSUM plane layout (q, pp, t): plane (p, q) in half hf=p//2 at
              # [q, p%2, :], so stage-W PSUM reads are contiguous 122-spans
              pys = [psy.tile([128, 8 * TW], f32, tag="py", bufs=PYB,
                              name=f"py{lob}_{ch}_{hf}_{_it}")
                     for hf in range(2)]
              pyv = [p[:].rearrange("o (q pp t) -> o q pp t", q=4, pp=2)
                     for p in pys]
              for pl in range(NP):
                  p_, q_ = pl // 4, pl % 4
                  out = pyv[p_ // 2][:, q_, p_ % 2, :]
                  for t in range(3):
                      st = 1 if t == 2 else 0
                      wv = lv if t == 1 else hv
                      for g in range(3):
                          nc.tensor.matmul(
                              out,
                              wv[:, pl, :, g, :],
                              ddv[g][:, st, :, pl, :],
                              start=(t == 0 and g == 0),
                              stop=(t == 2 and g == 2),
                              perf_mode=DR,
                          )
              # drain PSUM via fast ACT copies (frees banks quickly, keeps
              # the PE fed); all combining is then SBUF-only on DVE
              mbuf = vbuf.tile([128, 2 * 8 * TW], f32, tag="mbuf", bufs=2,
                               name=f"mb{lob}_{ch}_{_it}")
              nbuf = vbuf.tile([128, 8 * TW], f32, tag="nbuf", bufs=2,
                               name=f"nb{lob}_{ch}_{_it}")
              nv = nbuf[:].rearrange("o (p s t) -> o p s t", p=4, s=2)
              tbuf = vbuf.tile([128, 4 * TW], f32, tag="tbuf", bufs=2,
                               name=f"tb{lob}_{ch}_{_it}")
              for hf in range(2):
                  nc.scalar.activation(mbuf[:, hf * 8 * TW:(hf + 1) * 8 * TW],
                                       pys[hf][:], ACT.Copy)
              for hf in range(2):
                  mq = [mbuf[:, (hf * 8 + q * 2) * TW:(hf * 8 + q * 2 + 2) * TW]
                        for q in range(4)]
                  tq = tbuf[:, hf * 2 * TW:(hf + 1) * 2 * TW]
                  nq0 = nv[:, hf * 2:hf * 2 + 2, 0, :]
                  nq1 = nv[:, hf * 2:hf * 2 + 2, 1, :]
                  v2 = lambda ap: ap.rearrange("o (pp t) -> o pp t", pp=2)
                  nc.vector.tensor_tensor(out=tq, in0=mq[1], in1=mq[2],
                                          op=ALU.add)
                  nc.vector.tensor_tensor(out=nq0, in0=v2(tq), in1=v2(mq[0]),
                                          op=ALU.add)
                  nc.vector.tensor_tensor(out=tq, in0=mq[1], in1=mq[2],
                                          op=ALU.subtract)
                  nc.vector.tensor_tensor(out=nq1, in0=v2(tq), in1=v2(mq[3]),
                                          op=ALU.subtract)
              # stage H on Pool: y rows (x128 scaled; descaled on host)
              ysb = vbuf.tile([128, 2 * 128], f16, tag="ysb", bufs=4,
                              name=f"ys{lob}_{ch}_{_it}")
              yv = ysb[:].rearrange("o (r t s) -> o r s t", r=2, s=2)
              t2 = vbuf.tile([128, 2 * TW], f32, tag="t2", bufs=3,
                             name=f"t2{lob}_{ch}_{_it}")
              t2v = t2[:].rearrange("o (s t) -> o s t", s=2)
              ns = [nv[:, p] for p in range(4)]
              nc.gpsimd.tensor_tensor(out=t2v, in0=ns[1], in1=ns[2],
                                      op=ALU.add)
              nc.gpsimd.tensor_tensor(out=yv[:, 0, :, 0:60],
                                      in0=ns[0][:, :, 0:60],
                                      in1=t2v[:, :, 0:60], op=ALU.add)
              nc.gpsimd.tensor_tensor(out=t2v, in0=ns[1], in1=ns[2],
                                      op=ALU.subtract)
              nc.gpsimd.tensor_tensor(out=yv[:, 1, :, 0:60],
                                      in0=t2v[:, :, 0:60],
                                      in1=ns[3][:, :, 0:60], op=ALU.subtract)
              nc.sync.dma_start(
                  y_d[lob * 128:(lob + 1) * 128, 2 * ch:2 * ch + 2, :],
                  ysb[:])

          # gate all 3 lobs (paced by the expert DMA stream), then run the
          # conv chunk-major so each d chunk is read by all lobs and freed
          wts = [synth(lob) for lob in range(OBPC)]
          for ch in range(NT):
              for lob in range(OBPC):
                  conv_chunk(lob, ch, *wts[lob])
          ictx.close()

    nc.finalize()
    _prog_cache[iters] = nc
    return nc


def prepare_in_maps(x, convs, w_pw1, w_dw1, w_dw2, w_pw2):
    """Host-side slicing/layout prep. Returns list of 8 per-core input dicts."""
    import ml_dtypes

    x = np.asarray(x, dtype=F32)
    convs = np.asarray(convs, dtype=F32)
    w_pw1 = np.asarray(w_pw1, dtype=F32)
    w_dw1 = np.asarray(w_dw1, dtype=F32)
    w_dw2 = np.asarray(w_dw2, dtype=F32)
    w_pw2 = np.asarray(w_pw2, dtype=F32)
    e4m3 = ml_dtypes.float8_e4m3

    x0 = x[0]  # (768, 120, 120)

    # ---- Winograd transforms ----
    G1 = np.array([[1, 0, 0], [1, 1, 1], [1, -1, 1], [0, 0, 1]], dtype=F32)
    S1 = np.array([1, 0.5, 0.5, 1], dtype=F32)
    BT = np.array([[1, 0, -1, 0], [0, 1, 1, 0], [0, -1, 1, 0], [0, 1, 0, -1]],
                  dtype=F32)

    # input transform: d[c, r, s, p, q], 60x60 tiles, plane scales folded
    xpad = np.zeros((C, 122, 122), dtype=F32)
    xpad[:, 1:121, 1:121] = x0
    win = np.empty((C, 60, 60, 4, 4), dtype=F32)
    ridx = 2 * np.arange(60)
    for a in range(4):
        for b in range(4):
            win[:, :, :, a, b] = xpad[:, ridx[:, None] + a, ridx[None, :] + b]
    d = np.einsum("pa,crsab,qb->crspq", BT, win, BT)
    d *= (S1[:, None] * S1[None, :])
    d8 = d.astype(e4m3)
    dlo = (d - d8.astype(F32)).astype(e4m3)

    # experts pre-transformed, f16, x WSC: [e, o, plane, cin]
    cvw = np.einsum("pu,eoiuv,qv->eopqi", G1, convs, G1).reshape(E, C, NP, C)
    cvw = (cvw * WSC).astype(np.float16)

    # ---- routing-head constants (identical to the direct kernel) ----
    colscale = np.concatenate([
        np.full(C, 1.0 / (H * W), dtype=F32),
        np.full(C, 1.0 / 1600.0, dtype=F32),
        np.full(C, 1.0 / 576.0, dtype=F32),
    ])
    w1s = (w_pw1 * colscale[None, :]).astype(F32)
    w1t = np.ascontiguousarray(
        w1s.T.reshape(18, 128, S).transpose(1, 0, 2).reshape(128, 18 * S)
    )
    M35 = _bicubic_mat(3, 5)
    tkmm = np.ascontiguousarray(
        np.einsum("pi,qj->ijpq", M35, M35).reshape(9, 25)).astype(F32)
    sel = np.zeros((128, 64), dtype=F32)
    for b, terms in enumerate(_slot_terms(24, 5)):
        for (c_, s_) in terms:
            for z in range(5):
                sel[c_ * 16 + s_ * 8 + z, b * 5 + z] = 1.0
                sel[c_ * 16 + s_ * 8 + z, 25] = 1.0
    for b, terms in enumerate(_slot_terms(40, 3)):
        for (c_, s_) in terms:
            for z in range(3):
                sel[c_ * 16 + s_ * 8 + 5 + z, 32 + b * 3 + z] = 1.0
    sel = sel.astype(ml_dtypes.bfloat16)

    ones25 = np.ones((128, 25), dtype=F32)
    ident = np.eye(128, dtype=F32)
    wdw1 = np.ascontiguousarray(w_dw1.reshape(S, 9))
    wdw2 = np.ascontiguousarray(w_dw2.reshape(S, 9))

    in_maps = []
    for c in range(N_CORES):
        q, hh = c // 2, c % 2
        # d slice for this core's strips, packed [384, ch, st, i, pl, TW]
        dda = np.zeros((2, 384, NT, 2, NP, TW), dtype=e4m3)
        for st, src in ((0, d8), (1, dlo)):
            dq = src[:, 15 * q:15 * q + NT]        # (768, 15, 60, 4, 4)
            dv = dq.reshape(C, NT, 60, NP).transpose(0, 1, 3, 2)
            for g in range(3):
                for i in range(2):
                    blk = slice((g + 3 * i) * 128, (g + 3 * i + 1) * 128)
                    dda[st, g * 128:(g + 1) * 128, :, i, :, 0:60] = dv[blk]
        ddc = np.ascontiguousarray(
            dda.transpose(1, 2, 0, 3, 4, 5).reshape(384, NT * 2 * 2 * NP * TW))

        cvs = np.ascontiguousarray(
            cvw[:, 384 * hh:384 * (hh + 1)].reshape(E, 384, NP * C))

        xpt = np.ascontiguousarray(
            x0[:, PR * c:PR * (c + 1), :].transpose(2, 1, 0)
        ).astype(e4m3)
        maskmm = np.zeros((PR, W, 16), dtype=F32)
        for r_ in range(PR):
            grow = PR * c + r_
            for col in range(W):
                pc5, pc3 = col // 24, col // 40
                for s_ in range(2):
                    if grow // 24 == (PR * c) // 24 + s_:
                        maskmm[r_, col, s_ * 8 + pc5] = 1.0
                    if grow // 40 == (PR * c) // 40 + s_:
                        maskmm[r_, col, s_ * 8 + 5 + pc3] = 1.0
        maskmm = np.ascontiguousarray(
            maskmm.transpose(1, 0, 2)).astype(ml_dtypes.bfloat16)
        w2t = np.empty((S, 9 * 128), dtype=F32)
        for e in range(E):
            for lob in range(OBPC):
                rows = slice(e * C + (3 * hh + lob) * 128,
                             e * C + (3 * hh + lob) * 128 + 128)
                w2t[:, (e * 3 + lob) * 128:(e * 3 + lob + 1) * 128] = \
                    w_pw2[rows, :].T
        cf32 = np.concatenate([w1t, ones25, ident], axis=1)
        c48 = np.concatenate(
            [w2t, wdw1, wdw2,
             np.concatenate([tkmm, np.zeros((S - 9, 25), dtype=F32)], axis=0)],
            axis=1)
        in_maps.append({
            "dd": ddc, "xpt": xpt, "cvs": cvs, "cf32": cf32,
            "c48": c48, "selmm": sel, "maskmm": maskmm,
        })
    return in_maps


def reassemble(outs):
    """outs: list of 8 dicts with 'y_out' (384, 30, 128) f16 -> full f32."""
    y = np.empty((1, C, H, W), dtype=F32)
    for c in range(N_CORES):
        q, hh = c // 2, c % 2
        y[0, 384 * hh:384 * (hh + 1), 30 * q:30 * (q + 1), :] = \
            outs[c]["y_out"][:, :, 0:120].astype(F32) * (1.0 / WSC)
    return y


last_results = None  # BassKernelResults from the most recent run (for test.py)


def kernel(x, convs, w_pw1, w_dw1, w_dw2, w_pw2):
    global last_results
    from concourse import bass_utils

    nc = _get_program()
    in_maps = prepare_in_maps(x, convs, w_pw1, w_dw1, w_dw2, w_pw2)
    trace = bool(int(os.environ.get("KBENCH_TRACE", "0")))
    res = bass_utils.run_bass_kernel_spmd(
        nc, in_maps, core_ids=list(range(N_CORES)), trace=trace,
    )
    last_results = res
    return reassemble(res.results)


# revision 3
# speedup vs baseline: 1.0860x; 1.0352x over previous
# kernel2.py — DyResConv_Inf via F(2x2,3x3) Winograd on 8 TRN2 NeuronCores.
#
# Reference computation:
#   r = routing(x)                      # (3, 768) sigmoid gates from global pools of x
#   w = sum_e r[e,o] * convs[e,o,:,:,:] # fused 3x3 conv weight synthesis
#   y = conv2d(x, w, stride 1, pad 1)   # (1, 768, 120, 120)
#
# One SPMD program on 8 cores; core c (q = c//2 row-quarter, h = c%2
# channel-half) computes y for out-channels [384h, 384h+384) x rows
# [30q, 30q+30) (= Winograd tile strips [15q, 15q+15)).
#
# Routing head: identical to the direct-conv kernel (pool rows -> masked PE
# matmuls -> AllGather -> reconstruction + tiny net -> sigmoid gates).
#
# Weight synthesis: experts are pre-transformed on the host to the Winograd
# domain (scale-free G1 = [[1,0,0],[1,1,1],[1,-1,1],[0,0,1]]; the 1/2 plane
# scales are folded into the input transform), f16, x128. Device gates them
# per plane with diag(r) matmuls (output = transposed [cin, o] tiles), then
# ACT quantizes to fp8 e4m3 (wt_hi) and DVE computes + quantizes the
# residual (wt_lo).
#
# Main conv: 16 independent per-plane GEMMs in fp8 DoubleRow (cin blocks
# paired (g, g+3)), 3 streams accumulating into the same PSUM:
#   wt_hi @ d8  +  wt_lo @ d8  +  wt_hi @ dlo      (w_lo*d_lo dropped)
# d8/dlo are the host-side Winograd transform of x, split into e4m3 hi/lo.
# Per output chunk (= one tile strip: 2 out rows, 61 tile cols incl 1 junk),
# 16x9 DR matmuls accumulate m-planes in PSUM; the inverse transform
# (A1T = [[1,1,1,0],[0,1,-1,-1]]) runs as strided tensor_tensor ops on
# DVE (stage W, PSUM->SBUF) and Pool (stage H, SBUF->f16 y, x128 scaled;
# descaled on the host).
import os

import numpy as np

os.environ.setdefault("MYCRO_LOCAL_CACHE", "1")

N_CORES = 8
C = 768          # in = out channels
H = W = 120
S = 48           # squeeze channels
E = 3            # experts
NB = 6           # channel blocks of 128 (768/128)
OBPC = 3         # o-blocks per core
QR = 30          # output rows per quarter
PR = 15          # pool rows per core
NP = 16          # Winograd planes
NT = 15          # tile strips per core (2 out rows each)
TW = 61          # tile cols per strip incl 1 junk col
F32 = np.float32
WSC = 128.0      # weight scale folded into the f16 expert stream

N_JUNK = int(os.environ.get("KBENCH_JUNK", "0"))
N_WARM = int(os.environ.get("KBENCH_WARM", "0"))
GJUNK = int(os.environ.get("KBENCH_GJUNK", "0"))
QJUNK = int(os.environ.get("KBENCH_QJUNK", "0"))
PYB = int(os.environ.get("KBENCH_PYB", "3"))
CVB = int(os.environ.get("KBENCH_CVB", "2"))
JUNK4 = int(os.environ.get("KBENCH_JUNK4", "8"))
DDB = int(os.environ.get("KBENCH_DDB", "4"))


def _bicubic_mat(in_size, out_size):
    """PyTorch-style bicubic (a=-0.75), align_corners=False, border-replicate."""
    a = -0.75

    def k(x):
        x = abs(x)
        if x <= 1.0:
            return (a + 2) * x**3 - (a + 3) * x**2 + 1.0
        if x < 2.0:
            return a * x**3 - 5 * a * x**2 + 8 * a * x - 4 * a
        return 0.0

    M = np.zeros((out_size, in_size), dtype=F32)
    scale = in_size / out_size
    for j in range(out_size):
        src = (j + 0.5) * scale - 0.5
        i0 = int(np.floor(src))
        t = src - i0
        for m in range(-1, 3):
            idx = min(max(i0 + m, 0), in_size - 1)
            M[j, idx] += k(m - t)
    return M


def _slot_terms(blocksize, nblocks):
    """For each pool block: list of (core, slot) contributing partial sums."""
    terms = [[] for _ in range(nblocks)]
    for c in range(N_CORES):
        base = (PR * c) // blocksize
        for s in range(2):
            b = base + s
            if b >= nblocks:
                continue
            lo, hi = max(PR * c, blocksize * b), min(PR * c + PR, blocksize * (b + 1))
            if lo < hi:
                terms[b].append((c, s))
    return terms


_prog_cache = {}


def _get_program(iters=None):
    """Build (once per iters) the SPMD Bass/Tile program. Returns nc."""
    global _prog_cache
    if iters is None:
        iters = int(os.environ.get("KBENCH_ITERS", "1"))
    if iters in _prog_cache:
        return _prog_cache[iters]

    from contextlib import ExitStack

    import concourse.bass as bass
    import concourse.tile as tile
    from concourse import bacc, mybir

    f32 = mybir.dt.float32
    f16 = mybir.dt.float16
    fp8 = mybir.dt.float8e4
    bf16 = mybir.dt.bfloat16
    AX = mybir.AxisListType
    ALU = mybir.AluOpType
    ACT = mybir.ActivationFunctionType
    DR = mybir.MatmulPerfMode.DoubleRow

    nc = bacc.Bacc(
        "TRN2",
        target_bir_lowering=False,
        debug=False,
        enable_asserts=False,
        num_devices=N_CORES,
    )

    # ---- I/O tensors (per-core contents differ; shapes identical) ----
    # d: Winograd-transformed x, hi/lo fp8: [g-block rows, chunk, st, i, pl, TW]
    dd_d = nc.dram_tensor("dd", [384, NT * 2 * 2 * NP * TW], fp8,
                          kind="ExternalInput").ap()
    xpt_d = nc.dram_tensor("xpt", [W, PR, C], fp8, kind="ExternalInput").ap()
    # pre-transformed experts: [e, o(384), plane(16) x cin(768)] f16 x128
    cv_d = nc.dram_tensor("cvs", [E, 384, NP * C], f16, kind="ExternalInput").ap()
    cf32_d = nc.dram_tensor("cf32", [128, 18 * S + 25 + 128], f32,
                            kind="ExternalInput").ap()
    c48_d = nc.dram_tensor("c48", [S, 9 * 128 + 9 + 9 + 25], f32,
                           kind="ExternalInput").ap()
    sel_d = nc.dram_tensor("selmm", [128, 64], bf16, kind="ExternalInput").ap()
    mask_d = nc.dram_tensor("maskmm", [W, PR, 16], bf16,
                            kind="ExternalInput").ap()
    y_d = nc.dram_tensor("y_out", [384, QR, 128], f16, kind="ExternalOutput").ap()

    with tile.TileContext(nc) as tc, ExitStack() as ctx:
        consts = ctx.enter_context(tc.tile_pool(name="consts", bufs=1))
        small = ctx.enter_context(tc.tile_pool(name="small", bufs=1))
        dram = ctx.enter_context(tc.tile_pool(name="dram", bufs=1, space="DRAM"))
        cvp = ctx.enter_context(tc.tile_pool(name="cvp", bufs=2))
        ddp = ctx.enter_context(tc.tile_pool(name="ddp", bufs=2))
        for _it in range(iters):
          rctx = ExitStack()
          xpp = rctx.enter_context(tc.tile_pool(name=f"xpp{_it}", bufs=1))
          psr = rctx.enter_context(
              tc.tile_pool(name=f"psr{_it}", bufs=2, space="PSUM"))

          # ---- pool-critical transfers first on the sync queue ----
          maskmm = xpp.tile([W, PR * 16], bf16, tag="maskmm")
          nc.sync.dma_start(maskmm[:], mask_d)

          xpt_sb = xpp.tile([W, PR * C], fp8, tag="xpt")
          xptv = xpt_sb[:].rearrange("w (r c) -> w r c", r=PR)
          xpt_dmas = [
              nc.sync.dma_start(xptv[:, r0:r1, :], xpt_d[:, r0:r1, :])
              for r0, r1 in ((0, 4), (4, 8), (8, 12), (12, PR))
          ]

          # d chunks on the ACT queue (issued before any routing ACT work) so
          # cv-buffer stalls on the sync queue never block them; 3-strip
          # groups keep the DGE-gen cost small
          dd_tiles = {}
          dd_dmas = []
          CHB = 2 * 2 * NP * TW  # elems per (g, strip) per partition
          for grp in range(NT // 3):
              for g in range(3):
                  t = ddp.tile([128, 3 * CHB], fp8, tag=f"dd{g}",
                               bufs=2, name=f"dd{g}_{grp}_{_it}")
                  dd_dmas.append(nc.scalar.dma_start(
                      t[:], dd_d[g * 128:(g + 1) * 128,
                                 grp * 3 * CHB:(grp + 1) * 3 * CHB]))
                  dd_tiles[(grp, g)] = t

          cf32 = consts.tile([128, 18 * S + 25 + 128], f32, tag="cf32")
          w1t = cf32[:, 0:18 * S]
          ones25 = cf32[:, 18 * S:18 * S + 25]
          ident = cf32[:, 18 * S + 25:]
          c48 = consts.tile([S, 9 * 128 + 9 + 9 + 25], f32, tag="c48")
          w2t = c48[:, 0:9 * 128]
          wdw1 = c48[:, 9 * 128:9 * 128 + 9]
          wdw2 = c48[:, 9 * 128 + 9:9 * 128 + 18]
          tkmm = c48[0:9, 9 * 128 + 18:9 * 128 + 43]
          selmm = consts.tile([128, 64], bf16, tag="selmm")

          # PE warmup: ramp the clock gate to full p-state before the
          # latency-critical pool matmuls
          junkw = psr.tile([16, 128], f32, tag="pw", bufs=2, name=f"jw{_it}")
          for _w in range(N_WARM):
              nc.tensor.matmul(junkw[:], maskmm[:, 0:16], maskmm[:, 0:128],
                               start=True, stop=True, skip_group_check=True)

          # ---- stage-1 pools as PE matmuls: payload[pat, c] ----
          ppay = [psr.tile([16, 384], f32, tag="prt", bufs=2,
                           name=f"ppay{h2}_{_it}") for h2 in range(2)]
          for row in range(PR):
              for h2 in range(2):
                  nc.tensor.matmul(
                      ppay[h2][:],
                      maskmm[:, row * 16:(row + 1) * 16],
                      xptv[:, row, h2 * 384:(h2 + 1) * 384],
                      start=(row == 0), stop=(row == PR - 1),
                  )
          payload = xpp.tile([16, C], bf16, tag="payload")
          for h2 in range(2):
              nc.scalar.activation(payload[:, h2 * 384:(h2 + 1) * 384],
                                   ppay[h2][:], ACT.Copy)
          actwarm = small.tile([1, 1], f32, tag="actwarm")
          nc.scalar.activation(actwarm[:], ident[0:1, 0:1], ACT.Sigmoid)
          nc.scalar.activation(actwarm[:], ident[0:1, 0:1], ACT.Relu)
          nc.scalar.activation(actwarm[:], ident[0:1, 0:1], ACT.Copy,
                               scale=ident[0:1, 0:1])

          # ---- AllGather the slot sums ----
          cc_in = dram.tile([16, C], bf16, tag="ccin")
          cc_out = dram.tile([N_CORES, 16, C], bf16, tag="ccout",
                             addr_space="Shared")
          cc_dma = nc.scalar.dma_start(cc_in[:], payload[:])
          nc.gpsimd.collective_compute(
              "AllGather",
              ALU.bypass,
              replica_groups=[list(range(N_CORES))],
              ins=[cc_in[:].opt()],
              outs=[cc_out[:].opt()],
          )

          # gathered slot sums, (core,pat) on partitions
          g2pm = xpp.tile([128, C], bf16, tag="g2pm")
          nc.gpsimd.dma_start(
              g2pm[:], cc_out[:].rearrange("core pat c -> (core pat) c"))

          cf32_dma = nc.sync.dma_start(cf32[:], cf32_d)
          tile.add_dep_helper(cf32_dma.ins, xpt_dmas[3].ins, sync=True,
                              reason="consts after xpt")
          nc.sync.dma_start(c48[:], c48_d)
          nc.sync.dma_start(selmm[:], sel_d)

          # ---- expert-weight stream (sync queue, behind routing-critical
          # transfers): per (lob, e, plane-quad) tiles; then d chunks ----
          cv_tiles = {}
          cv_dmas = []
          for lob in range(OBPC):
              for pq in range(8):
                  for e in range(E):
                      t = cvp.tile([128, 2 * C], f16, tag=f"cv{e}",
                                   bufs=2 * CVB, name=f"cv{e}_{lob}_{pq}_{_it}")
                      cv_dmas.append(nc.sync.dma_start(
                          t[:],
                          cv_d[e, lob * 128:(lob + 1) * 128,
                               pq * 2 * C:(pq + 1) * 2 * C],
                      ))
                      cv_tiles[(lob, pq, e)] = t

          # dd transfers yield to the routing-critical + first expert loads;
          # late groups yield to the cv stream (gating is cv-paced)
          tile.add_dep_helper(dd_dmas[0].ins, xpt_dmas[3].ins, sync=True,
                              reason="dd grp0 after xpt")
          tile.add_dep_helper(dd_dmas[3].ins, cv_dmas[11].ins, sync=True,
                              reason="dd grp1 after cv l0")
          tile.add_dep_helper(dd_dmas[6].ins, cv_dmas[23].ins, sync=True,
                              reason="dd grp2 after cv l1")
          tile.add_dep_helper(dd_dmas[9].ins, cv_dmas[29].ins, sync=True,
                              reason="dd grp3 after cv l2q1")
          tile.add_dep_helper(dd_dmas[12].ins, cv_dmas[35].ins, sync=True,
                              reason="dd grp4 after cv l2q3")

          # junk matmuls bridge the collective wait
          junkb = psr.tile([16, 512], f32, tag="pw", bufs=2, name=f"jb{_it}")
          for _w in range(N_JUNK):
              nc.tensor.matmul(junkb[:], maskmm[:, 0:16], xpt_sb[:, 0:512],
                               start=True, stop=True, skip_group_check=True)

          # pool block sums via SEL matmuls
          stg = xpp.tile([32, C], f32, tag="stg")
          stg3 = xpp.tile([9, C], f32, tag="stg3")
          stgu = xpp.tile([25, C], f32, tag="stgu")
          for h2 in range(2):
              psel = psr.tile([32, 384], f32, tag="prt", bufs=2,
                              name=f"psel{h2}_{_it}")
              nc.tensor.matmul(psel[:], selmm[:][:, 0:32],
                               g2pm[:, h2 * 384:(h2 + 1) * 384],
                               start=True, stop=True)
              nc.scalar.activation(stg[:, h2 * 384:(h2 + 1) * 384], psel[:],
                                   ACT.Copy)
              psel3 = psr.tile([9, 384], f32, tag="prt", bufs=2,
                               name=f"psel3{h2}_{_it}")
              nc.tensor.matmul(psel3[:], selmm[:][:, 32:41],
                               g2pm[:, h2 * 384:(h2 + 1) * 384],
                               start=True, stop=True)
              nc.scalar.activation(stg3[:, h2 * 384:(h2 + 1) * 384], psel3[:],
                                   ACT.Copy)
          # bicubic 3->5 as a matmul in transposed space
          for h2 in range(2):
              pbic = psr.tile([25, 384], f32, tag="prt", bufs=2,
                              name=f"pbic{h2}_{_it}")
              nc.tensor.matmul(pbic[:], tkmm,
                               stg3[:, h2 * 384:(h2 + 1) * 384],
                               start=True, stop=True)
              nc.scalar.activation(stgu[:, h2 * 384:(h2 + 1) * 384], pbic[:],
                                   ACT.Copy)

          # transpose per pblock back to channel-major
          att = small.tile([128, 18 * 25], f32, tag="att")
          att5 = xpp.tile([128, NB * 32], f32, tag="att5")
          for p in range(NB):
              pt1 = psr.tile([128, 32], f32, tag="pw", bufs=2,
                             name=f"pt1_{p}_{_it}")
              nc.tensor.transpose(pt1[:], stg[:, p * 128:(p + 1) * 128],
                                  ident[0:32, 0:32])
              nc.scalar.activation(att5[:, p * 32:(p + 1) * 32], pt1[:], ACT.Copy)
              pt2 = psr.tile([128, 25], f32, tag="pw", bufs=2,
                             name=f"pt2_{p}_{_it}")
              nc.tensor.transpose(pt2[:], stgu[:, p * 128:(p + 1) * 128],
                                  ident[0:25, 0:25])
              nc.scalar.activation(att[:, (6 + p) * 25:(7 + p) * 25], pt2[:],
                                   ACT.Copy)
              nc.vector.tensor_scalar_mul(
                  att[:, p * 25:(p + 1) * 25], ones25,
                  att5[:, p * 32 + 25:p * 32 + 26])

          # ---- routing net ----
          ph = psr.tile([S, 25], f32, tag="prt", bufs=2)
          for j in range(18):
              rhs = (att5[:, (j - 12) * 32:(j - 12) * 32 + 25] if j >= 12
                     else att[:, j * 25:(j + 1) * 25])
              nc.tensor.matmul(
                  ph[:],
                  w1t[:, j * S:(j + 1) * S],
                  rhs,
                  start=(j == 0), stop=(j == 17),
              )
          hdd1 = xpp.tile([S, 25], f32, tag="hdd1")
          nc.scalar.activation(hdd1[:], ph[:], ACT.Relu)

          hdd2 = xpp.tile([S, 9], f32, tag="hdd2")
          h1v = hdd1[:].rearrange("s (p q) -> s p q", p=5)
          for uv in range(9):
              u, v = uv // 3, uv % 3
              if uv == 0:
                  nc.vector.tensor_scalar_mul(
                      hdd2[:].rearrange("s (p q) -> s p q", p=3),
                      h1v[:, u:u + 3, v:v + 3], wdw1[:, 0:1]
                  )
              else:
                  t9 = xpp.tile([S, 9], f32, tag="t9", name=f"t9_{uv}")
                  nc.vector.tensor_scalar_mul(
                      t9[:].rearrange("s (p q) -> s p q", p=3),
                      h1v[:, u:u + 3, v:v + 3], wdw1[:, uv:uv + 1]
                  )
                  nc.vector.tensor_tensor(
                      out=hdd2[:], in0=hdd2[:], in1=t9[:], op=ALU.add
                  )
          nc.scalar.activation(hdd2[:], hdd2[:], ACT.Relu)

          t9b = xpp.tile([S, 9], f32, tag="t9b")
          nc.vector.tensor_tensor(out=t9b[:], in0=hdd2[:], in1=wdw2[:], op=ALU.mult)
          hdd3 = xpp.tile([S, 1], f32, tag="hdd3")
          nc.vector.tensor_reduce(hdd3[:], t9b[:], axis=AX.X, op=ALU.add)
          nc.scalar.activation(hdd3[:], hdd3[:], ACT.Relu)

          for _w in range(JUNK4):
              nc.tensor.matmul(junkb[:], maskmm[:, 0:16], xpt_sb[:, 0:512],
                               start=True, stop=True, skip_group_check=True)
          pr = psr.tile([128, 9], f32, tag="prt", bufs=2)
          for m in range(9):
              nc.tensor.matmul(
                  pr[:, m:m + 1],
                  w2t[:, m * 128:(m + 1) * 128],
                  hdd3[:],
                  start=True, stop=True, skip_group_check=True,
              )
          r_sb = small.tile([128, 9], f32, tag="r_sb")
          nc.scalar.activation(r_sb[:], pr[:], ACT.Sigmoid)

          # f16 diag-gate matrices for PE-side synthesis (col = e*3+lob)
          dall = small.tile([128, 9 * 128], f16, tag="dall")
          for col in range(9):
              nc.vector.tensor_scalar_mul(
                  dall[:, col * 128:(col + 1) * 128], ident[:],
                  r_sb[:, col:col + 1]
              )

          # bridge the routing-net serial tail (ACT/DVE ping-pong to gates)
          for _w in range(GJUNK):
              nc.tensor.matmul(junkb[:], maskmm[:, 0:16], xpt_sb[:, 0:512],
                               start=True, stop=True, skip_group_check=True)

          rctx.close()  # free routing-phase SBUF/PSUM before the conv phase
          ictx = ExitStack()
          wtp = ictx.enter_context(tc.tile_pool(name=f"wtp{_it}", bufs=3))
          vbuf = ictx.enter_context(tc.tile_pool(name=f"vbuf{_it}", bufs=2))
          psy = ictx.enter_context(
              tc.tile_pool(name=f"psy{_it}", bufs=4, space="PSUM"))
          psw = ictx.enter_context(
              tc.tile_pool(name=f"psw{_it}", bufs=2, space="PSUM"))

          junkg = (psy.tile([16, 128], f32, tag="junkg", bufs=1,
                            name=f"jg{_it}") if QJUNK else None)

          def synth(lob):
              """Gate the pre-transformed experts for o-block lob; fp8 split.
              Weight layout: [cin-in-block, (plane, i, g, o)]. Junk matmuls
              after each quad keep the PE p-state hot across cv-DMA stalls."""
              wthi = wtp.tile([128, NP * 2 * 3 * 128], fp8, tag="wthi",
                              name=f"wthi{lob}_{_it}")
              wtlo = wtp.tile([128, NP * 2 * 3 * 128], fp8, tag="wtlo",
                              name=f"wtlo{lob}_{_it}")
              hv = wthi[:].rearrange("c (pl b o) -> c pl b o", pl=NP, b=6)
              lv = wtlo[:].rearrange("c (pl b o) -> c pl b o", pl=NP, b=6)
              for pl in range(NP):
                  cvq = [cv_tiles[(lob, pl // 2, e)][:].rearrange(
                      "o (p c) -> o p c", p=2) for e in range(E)]
                  pw = psw.tile([128, 768], f32, tag="pws", bufs=2,
                                name=f"pw{lob}_{pl}_{_it}")
                  for b in range(6):
                      for e in range(E):
                          nc.tensor.matmul(
                              pw[:, b * 128:(b + 1) * 128],
                              cvq[e][:, pl % 2, b * 128:(b + 1) * 128],
                              dall[:, (e * 3 + lob) * 128:
                                   (e * 3 + lob + 1) * 128],
                              start=(e == 0), stop=(e == E - 1),
                              skip_group_check=True,
                          )
                  hi_blk = hv[:, pl]
                  nc.scalar.activation(hi_blk, pw[:], ACT.Copy)
                  nc.vector.tensor_tensor(out=lv[:, pl], in0=pw[:],
                                          in1=hi_blk, op=ALU.subtract)
                  if pl % 4 == 3:
                      for _w in range(QJUNK):
                          nc.tensor.matmul(junkg[:], dall[:, 0:16],
                                           dall[:, 0:128], start=True,
                                           stop=True, skip_group_check=True)
              return wthi, wtlo

          def conv_chunk(lob, ch, wthi, wtlo):
              """One tile strip of the Winograd conv for o-block lob: 16
              plane-GEMMs in fp8 DR + inverse transform (DVE stage-W,
              Pool stage-H)."""
              hv = wthi[:].rearrange("c (pl i g o) -> c pl i g o",
                                     pl=NP, i=2, g=3)
              lv = wtlo[:].rearrange("c (pl i g o) -> c pl i g o",
                                     pl=NP, i=2, g=3)
              ddv = [dd_tiles[(ch // 3, g)][:,
                                            (ch % 3) * CHB:(ch % 3 + 1) * CHB]
                     .rearrange("c (st i pl t) -> c st i pl t", st=2, i=2,
                                pl=NP)
                     for g in range(3)]
              # PSUM plane layout (q, pp, t): plane (p, q) in half hf=p//2 at
              # [q, p%2, :], so stage-W PSUM reads are contiguous 122-spans
              pys = [psy.tile([128, 8 * TW], f32, tag="py", bufs=PYB,
                              name=f"py{lob}_{ch}_{hf}_{_it}")
                     for hf in range(2)]
              pyv = [p[:].rearrange("o (q pp t) -> o q pp t", q=4, pp=2)
                     for p in pys]
              for pl in range(NP):
                  p_, q_ = pl // 4, pl % 4
                  out = pyv[p_ // 2][:, q_, p_ % 2, :]
                  for t in range(3):
                      st = 1 if t == 2 else 0
                      wv = lv if t == 1 else hv
                      for g in range(3):
                          nc.tensor.matmul(
                              out,
                              wv[:, pl, :, g, :],
                              ddv[g][:, st, :, pl, :],
                              start=(t == 0 and g == 0),
                              stop=(t == 2 and g == 2),
                              perf_mode=DR,
                          )
              # drain PSUM via fast ACT copies (frees banks quickly, keeps
              # the PE fed); all combining is then SBUF-only on DVE
              mbuf = vbuf.tile([128, 2 * 8 * TW], f32, tag="mbuf", bufs=2,
                               name=f"mb{lob}_{ch}_{_it}")
              nbuf = vbuf.tile([128, 8 * TW], f32, tag="nbuf", bufs=2,
                               name=f"nb{lob}_{ch}_{_it}")
              nv = nbuf[:].rearrange("o (p s t) -> o p s t", p=4, s=2)
              tbuf = vbuf.tile([128, 4 * TW], f32, tag="tbuf", bufs=2,
                               name=f"tb{lob}_{ch}_{_it}")
              for hf in range(2):
                  nc.scalar.activation(mbuf[:, hf * 8 * TW:(hf + 1) * 8 * TW],
                                       pys[hf][:], ACT.Copy)
              for hf in range(2):
                  mq = [mbuf[:, (hf * 8 + q * 2) * TW:(hf * 8 + q * 2 + 2) * TW]
                        for q in range(4)]
                  tq = tbuf[:, hf * 2 * TW:(hf + 1) * 2 * TW]
                  nq0 = nv[:, hf * 2:hf * 2 + 2, 0, :]
                  nq1 = nv[:, hf * 2:hf * 2 + 2, 1, :]
                  v2 = lambda ap: ap.rearrange("o (pp t) -> o pp t", pp=2)
                  nc.vector.tensor_tensor(out=tq, in0=mq[1], in1=mq[2],
                                          op=ALU.add)
                  nc.vector.tensor_tensor(out=nq0, in0=v2(tq), in1=v2(mq[0]),
                                          op=ALU.add)
                  nc.vector.tensor_tensor(out=tq, in0=mq[1], in1=mq[2],
                                          op=ALU.subtract)
                  nc.vector.tensor_tensor(out=nq1, in0=v2(tq), in1=v2(mq[3]),
                                          op=ALU.subtract)
              # stage H on Pool: y rows (x128 scaled; descaled on host)
              ysb = vbuf.tile([128, 2 * 128], f16, tag="ysb", bufs=4,
                              name=f"ys{lob}_{ch}_{_it}")
              yv = ysb[:].rearrange("o (r t s) -> o r s t", r=2, s=2)
              t2 = vbuf.tile([128, 2 * TW], f32, tag="t2", bufs=3,
                             name=f"t2{lob}_{ch}_{_it}")
              t2v = t2[:].rearrange("o (s t) -> o s t", s=2)
              ns = [nv[:, p] for p in range(4)]
              nc.gpsimd.tensor_tensor(out=t2v, in0=ns[1], in1=ns[2],
                                      op=ALU.add)
              nc.gpsimd.tensor_tensor(out=yv[:, 0, :, 0:60],
                                      in0=ns[0][:, :, 0:60],
                                      in1=t2v[:, :, 0:60], op=ALU.add)
              nc.gpsimd.tensor_tensor(out=t2v, in0=ns[1], in1=ns[2],
                                      op=ALU.subtract)
              nc.gpsimd.tensor_tensor(out=yv[:, 1, :, 0:60],
                                      in0=t2v[:, :, 0:60],
                                      in1=ns[3][:, :, 0:60], op=ALU.subtract)
              nc.sync.dma_start(
                  y_d[lob * 128:(lob + 1) * 128, 2 * ch:2 * ch + 2, :],
                  ysb[:])

          # gate all 3 lobs (paced by the expert DMA stream), then run the
          # conv chunk-major so each d chunk is read by all lobs and freed
          wts = [synth(lob) for lob in range(OBPC)]
          for ch in range(NT):
              for lob in range(OBPC):
                  conv_chunk(lob, ch, *wts[lob])
          ictx.close()

    nc.finalize()
    _prog_cache[iters] = nc
    return nc


def prepare_in_maps(x, convs, w_pw1, w_dw1, w_dw2, w_pw2):
    """Host-side slicing/layout prep. Returns list of 8 per-core input dicts."""
    import ml_dtypes

    x = np.asarray(x, dtype=F32)
    convs = np.asarray(convs, dtype=F32)
    w_pw1 = np.asarray(w_pw1, dtype=F32)
    w_dw1 = np.asarray(w_dw1, dtype=F32)
    w_dw2 = np.asarray(w_dw2, dtype=F32)
    w_pw2 = np.asarray(w_pw2, dtype=F32)
    e4m3 = ml_dtypes.float8_e4m3

    x0 = x[0]  # (768, 120, 120)

    # ---- Winograd transforms ----
    G1 = np.array([[1, 0, 0], [1, 1, 1], [1, -1, 1], [0, 0, 1]], dtype=F32)
    S1 = np.array([1, 0.5, 0.5, 1], dtype=F32)
    BT = np.array([[1, 0, -1, 0], [0, 1, 1, 0], [0, -1, 1, 0], [0, 1, 0, -1]],
                  dtype=F32)

    # input transform: d[c, r, s, p, q], 60x60 tiles, plane scales folded
    xpad = np.zeros((C, 122, 122), dtype=F32)
    xpad[:, 1:121, 1:121] = x0
    win = np.empty((C, 60, 60, 4, 4), dtype=F32)
    ridx = 2 * np.arange(60)
    for a in range(4):
        for b in range(4):
            win[:, :, :, a, b] = xpad[:, ridx[:, None] + a, ridx[None, :] + b]
    d = np.einsum("pa,crsab,qb->crspq", BT, win, BT)
    d *= (S1[:, None] * S1[None, :])
    d8 = d.astype(e4m3)
    dlo = (d - d8.astype(F32)).astype(e4m3)

    # experts pre-transformed, f16, x WSC: [e, o, plane, cin]
    cvw = np.einsum("pu,eoiuv,qv->eopqi", G1, convs, G1).reshape(E, C, NP, C)
    cvw = (cvw * WSC).astype(np.float16)

    # ---- routing-head constants (identical to the direct kernel) ----
    colscale = np.concatenate([
        np.full(C, 1.0 / (H * W), dtype=F32),
        np.full(C, 1.0 / 1600.0, dtype=F32),
        np.full(C, 1.0 / 576.0, dtype=F32),
    ])
    w1s = (w_pw1 * colscale[None, :]).astype(F32)
    w1t = np.ascontiguousarray(
        w1s.T.reshape(18, 128, S).transpose(1, 0, 2).reshape(128, 18 * S)
    )
    M35 = _bicubic_mat(3, 5)
    tkmm = np.ascontiguousarray(
        np.einsum("pi,qj->ijpq", M35, M35).reshape(9, 25)).astype(F32)
    sel = np.zeros((128, 64), dtype=F32)
    for b, terms in enumerate(_slot_terms(24, 5)):
        for (c_, s_) in terms:
            for z in range(5):
                sel[c_ * 16 + s_ * 8 + z, b * 5 + z] = 1.0
                sel[c_ * 16 + s_ * 8 + z, 25] = 1.0
    for b, terms in enumerate(_slot_terms(40, 3)):
        for (c_, s_) in terms:
            for z in range(3):
                sel[c_ * 16 + s_ * 8 + 5 + z, 32 + b * 3 + z] = 1.0
    sel = sel.astype(ml_dtypes.bfloat16)

    ones25 = np.ones((128, 25), dtype=F32)
    ident = np.eye(128, dtype=F32)
    wdw1 = np.ascontiguousarray(w_dw1.reshape(S, 9))
    wdw2 = np.ascontiguousarray(w_dw2.reshape(S, 9))

    in_maps = []
    for c in range(N_CORES):
        q, hh = c // 2, c % 2
        # d slice for this core's strips, packed [384, ch, st, i, pl, TW]
        dda = np.zeros((2, 384, NT, 2, NP, TW), dtype=e4m3)
        for st, src in ((0, d8), (1, dlo)):
            dq = src[:, 15 * q:15 * q + NT]        # (768, 15, 60, 4, 4)
            dv = dq.reshape(C, NT, 60, NP).transpose(0, 1, 3, 2)
            for g in range(3):
                for i in range(2):
                    blk = slice((g + 3 * i) * 128, (g + 3 * i + 1) * 128)
                    dda[st, g * 128:(g + 1) * 128, :, i, :, 0:60] = dv[blk]
        ddc = np.ascontiguousarray(
            dda.transpose(1, 2, 0, 3, 4, 5).reshape(384, NT * 2 * 2 * NP * TW))

        cvs = np.ascontiguousarray(
            cvw[:, 384 * hh:384 * (hh + 1)].reshape(E, 384, NP * C))

        xpt = np.ascontiguousarray(
            x0[:, PR * c:PR * (c + 1), :].transpose(2, 1, 0)
        ).astype(e4m3)
        maskmm = np.zeros((PR, W, 16), dtype=F32)
        for r_ in range(PR):
            grow = PR * c + r_
            for col in range(W):
                pc5, pc3 = col // 24, col // 40
                for s_ in range(2):
                    if grow // 24 == (PR * c) // 24 + s_:
                        maskmm[r_, col, s_ * 8 + pc5] = 1.0
                    if grow // 40 == (PR * c) // 40 + s_:
                        maskmm[r_, col, s_ * 8 + 5 + pc3] = 1.0
        maskmm = np.ascontiguousarray(
            maskmm.transpose(1, 0, 2)).astype(ml_dtypes.bfloat16)
        w2t = np.empty((S, 9 * 128), dtype=F32)
        for e in range(E):
            for lob in range(OBPC):
                rows = slice(e * C + (3 * hh + lob) * 128,
                             e * C + (3 * hh + lob) * 128 + 128)
                w2t[:, (e * 3 + lob) * 128:(e * 3 + lob + 1) * 128] = \
                    w_pw2[rows, :].T
        cf32 = np.concatenate([w1t, ones25, ident], axis=1)
        c48 = np.concatenate(
            [w2t, wdw1, wdw2,
             np.concatenate([tkmm, np.zeros((S - 9, 25), dtype=F32)], axis=0)],
            axis=1)
        in_maps.append({
            "dd": ddc, "xpt": xpt, "cvs": cvs, "cf32": cf32,
            "c48": c48, "selmm": sel, "maskmm": maskmm,
        })
    return in_maps


def reassemble(outs):
    """outs: list of 8 dicts with 'y_out' (384, 30, 128) f16 -> full f32."""
    y = np.empty((1, C, H, W), dtype=F32)
    for c in range(N_CORES):
        q, hh = c // 2, c % 2
        y[0, 384 * hh:384 * (hh + 1), 30 * q:30 * (q + 1), :] = \
            outs[c]["y_out"][:, :, 0:120].astype(F32) * (1.0 / WSC)
    return y


last_results = None  # BassKernelResults from the most recent run (for test.py)


def kernel(x, convs, w_pw1, w_dw1, w_dw2, w_pw2):
    global last_results
    from concourse import bass_utils

    nc = _get_program()
    in_maps = prepare_in_maps(x, convs, w_pw1, w_dw1, w_dw2, w_pw2)
    trace = bool(int(os.environ.get("KBENCH_TRACE", "0")))
    res = bass_utils.run_bass_kernel_spmd(
        nc, in_maps, core_ids=list(range(N_CORES)), trace=trace,
    )
    last_results = res
    return reassemble(res.results)
